# revision 37
# baseline (speedup 1.0000x reference)
"""Trainium2 Bass kernel for the LIDAR2D 4-direction selective-scan block.

Sharding: (batch, d_inner/4). Core c handles batch c//4 and a 128-channel
slice of d_inner (E=512): the host passes x[b] and channel-sliced scan
params per core, so the program stays SPMD-identical. Each core computes
the full-E front for its batch once, then scans its 128 channels as two
64-channel groups (cg) x 16 states x 4 directions, and emits a partial
out-projection (Dm, L). The host sums 4 partials per batch.

Scan layout per core: tiles of [128 partitions = (nsub in {0,1}) x (64
channels), free = L] — 8 tiles j=0..7 cover states n = 2j + nsub. The
recurrence h_t = exp(A*delta_t)*h_{t-1} + delta_t*B_t*u_t runs in a single
DVE tensor_tensor_scan per tile (2 cyc/elem — the hardware floor and the
kernel's critical path). y_t = C_t . h_t is a PE matmul with a 0/1
selection matrix contracting the two nsub rows per channel, accumulated
over j in PSUM. Direction permutations (reverse / spatial transpose) are
pure access-pattern tricks on reads/writes.

Perf structure: the front runs in bf16 (1 PE pass/col) with per-512-chunk
transient tiles. With one batch per core there is no second front to
overlap; the DVE stream is the 64 scans plus their elementwise feeds, and
the PE/Act fronts run only in the ~100us head.
"""

import os
import sys

for _p in ("/opt/trn_rl_repo", os.path.expanduser("~/.axon_site/_ro/trn_rl_repo")):
    if os.path.isdir(_p) and _p not in sys.path:
        sys.path.insert(0, _p)

import numpy as np
import ml_dtypes

import concourse.bass as bass
import concourse.bacc as bacc
import concourse.mybir as mybir
from concourse.tile import TileContext
from concourse.bass_utils import run_bass_kernel_spmd

F32 = mybir.dt.float32
F32R = mybir.dt.float32r
BF16 = mybir.dt.bfloat16
AF = mybir.ActivationFunctionType
OP = mybir.AluOpType

# Problem shape (hardcoded per the harness contract).
B, L, DM, E, N, R, MID, H, W = 2, 2304, 256, 512, 16, 16, 32, 48, 48
NCORES = 8
ESH = E // NCORES          # 64 channels per core
NJ = N // 2                # 8 scan tiles per (b, k); rows = (nsub, e_loc)
HALF = L // 2              # 1152, for PSUM-sized y accumulation

TRACE = bool(os.environ.get("KERNEL_TRACE"))
_LAST_EXEC_NS = None


def _install_profile_shim():
    """Make run_bass_kernel_spmd(trace=True) work in this container:
    register the NTFF hook (antenv.axon_hooks is absent here) and stub
    the S3 artifact upload."""
    import types
    try:
        from antenv.axon_hooks import get_axon_ntff_profile_hook  # noqa: F401
    except ImportError:
        import antenv
        mod = types.ModuleType("antenv.axon_hooks")
        mod._HOOK = None
        mod.set_axon_ntff_profile_hook = lambda h: setattr(mod, "_HOOK", h)
        mod.get_axon_ntff_profile_hook = lambda: mod._HOOK
        sys.modules["antenv.axon_hooks"] = mod
        antenv.axon_hooks = mod
        try:
            from trn_agent_boot.trn_boot import _ntff_profile_via_ctypes
            hook = _ntff_profile_via_ctypes("/opt/axon/libaxon_pjrt.so")
            if hook is not None:
                mod._HOOK = hook
        except Exception as e:  # pragma: no cover
            print(f"profile shim: hook install failed: {e}")
    import concourse.bass_utils as bu
    bu.upload_artifacts = lambda tmpdir: f"file://{tmpdir}"


def _chunks(total, step):
    out = []
    c0 = 0
    while c0 < total:
        out.append((c0, min(step, total - c0)))
        c0 += step
    return out


MM_CHUNKS = _chunks(L, 512)          # matmul free-dim chunks over full L
MM_CHUNKS_HALF = _chunks(HALF, 512)  # chunks within a 1152 half


class Env:
    pass


def build_program():
    nc = bacc.Bacc()
    g = Env()
    g.nc = nc

    # ---- DRAM parameters (same shapes on every core; values differ) ----
    g.xT_d = nc.declare_dram_parameter("xT", [DM, L], BF16, isOutput=False)
    g.w_in_d = nc.declare_dram_parameter("w_in", [DM, E + 2 * ESH], BF16, isOutput=False)
    g.w_pw1_d = nc.declare_dram_parameter("w_pw1", [E, MID], BF16, isOutput=False)
    g.pw1b_d = nc.declare_dram_parameter("pw1b", [MID, 1], F32, isOutput=False)
    g.dwtap_d = nc.declare_dram_parameter("dwtap", [MID, 9], F32, isOutput=False)
    g.w_pw2_d = nc.declare_dram_parameter("w_pw2", [MID, E], BF16, isOutput=False)
    g.w_xp_d = nc.declare_dram_parameter("w_xp", [E, 64], BF16, isOutput=False)
    g.w_dt_d = nc.declare_dram_parameter("w_dt", [R, 2 * ESH], BF16, isOutput=False)
    g.spb_d = nc.declare_dram_parameter("spb", [2, ESH, 1], F32, isOutput=False)
    g.ascale_d = nc.declare_dram_parameter("ascale", [2, 2 * ESH, NJ], F32, isOutput=False)
    g.dire_d = nc.declare_dram_parameter("dire", [2, ESH, 4], F32, isOutput=False)
    g.dp4_d = nc.declare_dram_parameter("dp4", [2, ESH, 1], F32, isOutput=False)
    g.dpb_d = nc.declare_dram_parameter("dpb", [2, ESH, 1], F32, isOutput=False)
    g.w_out_d = nc.declare_dram_parameter("w_out", [2, ESH, DM], BF16, isOutput=False)
    g.sel_d = nc.declare_dram_parameter("sel", [2 * ESH, ESH], BF16, isOutput=False)
    g.out_d = nc.declare_dram_parameter("out", [DM, L], F32, isOutput=True)

    with TileContext(nc) as tc:
        g.tc = tc
        with tc.tile_pool(name="const", bufs=1) as cp, \
             tc.tile_pool(name="persist", bufs=1) as pp, \
             tc.tile_pool(name="front", bufs=1) as fp, \
             tc.tile_pool(name="scan", bufs=1) as sp, \
             tc.tile_pool(name="work", bufs=2) as wp, \
             tc.tile_pool(name="fpsum", bufs=2, space="PSUM") as fps, \
             tc.tile_pool(name="ypsum", bufs=1, space="PSUM") as yps, \
             tc.tile_pool(name="bounce", bufs=1, space="DRAM") as bp:
            g.cp, g.pp, g.fp, g.sp, g.wp, g.fps, g.yps, g.bp = \
                cp, pp, fp, sp, wp, fps, yps, bp
            _load_consts(g)

            # persistent per-b products of the front
            g.xc_sl = [pp.tile([ESH, L], BF16, tag=f"xc_sl{b}", name=f"xc_sl{b}") for b in range(B)]
            g.delta_rep = [pp.tile([128, L], F32, tag=f"drep{b}", name=f"drep{b}") for b in range(B)]
            g.z_sl = [pp.tile([ESH, L], BF16, tag=f"z{b}", name=f"z{b}") for b in range(B)]
            g.y_acc = [pp.tile([ESH, L], F32, tag=f"yacc{b}", name=f"yacc{b}") for b in range(B)]
            g.bsrc = bp.tile([N, L], BF16, tag="bsrc", name="bsrc")
            g.csrc = bp.tile([N, L], BF16, tag="csrc", name="csrc")
            # h12/acc full-L per-b (conv needs the whole plane)
            g.h12 = fp.tile([MID, L], BF16, tag="h12", name="h12")
            g.accB = fp.tile([MID, L], BF16, tag="accB", name="accB")
            g.xdbl = fp.tile([64, L], BF16, tag="xdbl", name="xdbl")
            g.dtlin = [fp.tile([ESH, L], BF16, tag=f"dtlin{c}", name=f"dtlin{c}") for c in range(2)]
            g.dA = [None] * NJ
            g.yv = [None, None]

            # ---- emission schedule (one batch per core, two cg groups) ----
            _front_stageA(g)
            _front_conv(g)
            _front_stageB(g)
            _scan_prep(g, 0)
            du = _mk_du(g, 0, 0)
            pend, du = _scan_k(g, 0, 0, du,
                               prefetch=lambda: _mk_du(g, 0, 1))
            _stage_z(g)
            for k in range(1, 4):
                nxt = ((lambda kk=k: (lambda: _mk_du(g, 0, kk + 1)))()
                       if k < 3 else (lambda: _mk_du(g, 1, 0)))
                pend, du = _scan_k(g, 0, k, du, pending=pend, prefetch=nxt)
            _scan_prep(g, 1)
            pend()
            _finish_yv(g, 0)
            pend = None
            for k in range(4):
                nxt = ((lambda kk=k: (lambda: _mk_du(g, 1, kk + 1)))()
                       if k < 3 else None)
                pend, du = _scan_k(g, 1, k, du, pending=pend, prefetch=nxt)
            pend()
            _finish_yv(g, 1)
            _out_proj(g)

    nc.finalize()
    return nc


def _load_consts(g):
    nc, cp = g.nc, g.cp
    g.w_in_t = [cp.tile([128, E + 2 * ESH], BF16, tag=f"w_in{t}", name=f"w_in{t}") for t in range(2)]
    for t in range(2):
        nc.sync.dma_start(out=g.w_in_t[t][:], in_=g.w_in_d[t * 128:(t + 1) * 128, :])
    g.w_pw1_t = [cp.tile([128, MID], BF16, tag=f"w_pw1{t}", name=f"w_pw1{t}") for t in range(4)]
    for t in range(4):
        nc.sync.dma_start(out=g.w_pw1_t[t][:], in_=g.w_pw1_d[t * 128:(t + 1) * 128, :])
    g.pw1b_t = cp.tile([MID, 1], F32, tag="pw1b", name="pw1b")
    nc.sync.dma_start(out=g.pw1b_t[:], in_=g.pw1b_d[:])
    g.dwtap_t = cp.tile([MID, 9], F32, tag="dwtap", name="dwtap")
    nc.sync.dma_start(out=g.dwtap_t[:], in_=g.dwtap_d[:])
    g.w_pw2_t = cp.tile([MID, E], BF16, tag="w_pw2", name="w_pw2")
    nc.sync.dma_start(out=g.w_pw2_t[:], in_=g.w_pw2_d[:])
    g.w_xp_t = [cp.tile([128, 64], BF16, tag=f"w_xp{t}", name=f"w_xp{t}") for t in range(4)]
    for t in range(4):
        nc.sync.dma_start(out=g.w_xp_t[t][:], in_=g.w_xp_d[t * 128:(t + 1) * 128, :])
    g.w_dt_t = cp.tile([R, 2 * ESH], BF16, tag="w_dt", name="w_dt")
    nc.sync.dma_start(out=g.w_dt_t[:], in_=g.w_dt_d[:])
    g.spb_t = [cp.tile([ESH, 1], F32, tag=f"spb{c}", name=f"spb{c}") for c in range(2)]
    g.ascale_t = [cp.tile([2 * ESH, NJ], F32, tag=f"ascale{c}", name=f"ascale{c}") for c in range(2)]
    g.dire_t = [cp.tile([ESH, 4], F32, tag=f"dire{c}", name=f"dire{c}") for c in range(2)]
    g.dp4_t = [cp.tile([ESH, 1], F32, tag=f"dp4{c}", name=f"dp4{c}") for c in range(2)]
    g.dpb_t = [cp.tile([ESH, 1], F32, tag=f"dpb{c}", name=f"dpb{c}") for c in range(2)]
    g.w_out_t = [cp.tile([ESH, DM], BF16, tag=f"w_out{c}", name=f"w_out{c}") for c in range(2)]
    for c in range(2):
        nc.sync.dma_start(out=g.spb_t[c][:], in_=g.spb_d[c])
        nc.sync.dma_start(out=g.ascale_t[c][:], in_=g.ascale_d[c])
        nc.sync.dma_start(out=g.dire_t[c][:], in_=g.dire_d[c])
        nc.sync.dma_start(out=g.dp4_t[c][:], in_=g.dp4_d[c])
        nc.sync.dma_start(out=g.dpb_t[c][:], in_=g.dpb_d[c])
        nc.sync.dma_start(out=g.w_out_t[c][:], in_=g.w_out_d[c])
    g.sel_t = cp.tile([2 * ESH, ESH], BF16, tag="sel", name="sel")
    nc.sync.dma_start(out=g.sel_t[:], in_=g.sel_d[:])


def _front_stageA(g):
    """x+pos -> xz -> (z slices, xh) -> pw1 -> h12 (full [MID, L])."""
    nc, fp, fps = g.nc, g.fp, g.fps
    for (c0, nf) in MM_CHUNKS:
        xin = [fp.tile([128, 512], BF16, tag=f"xin{t}", bufs=1, name=f"xin{t}") for t in range(2)]
        for t in range(2):
            nc.sync.dma_start(out=xin[t][:, :nf],
                              in_=g.xT_d[t * 128:(t + 1) * 128, c0:c0 + nf])
        xhc = [fp.tile([128, 512], BF16, tag=f"xhc{m}", bufs=2, name=f"xhc{m}") for m in range(4)]
        for m in range(4):
            ps = fps.tile([128, 512], F32, tag="fps", bufs=2, name="ps_xz")
            for kt in range(2):
                nc.tensor.matmul(ps[:, :nf],
                                 lhsT=g.w_in_t[kt][:, m * 128:(m + 1) * 128],
                                 rhs=xin[kt][:, :nf],
                                 start=(kt == 0), stop=(kt == 1))
            nc.scalar.activation(out=xhc[m][:, :nf], in_=ps[:, :nf], func=AF.Copy)
        ps1 = fps.tile([MID, 512], F32, tag="fps", bufs=2, name="ps_pw1")
        for kt in range(4):
            nc.tensor.matmul(ps1[:, :nf], lhsT=g.w_pw1_t[kt][:],
                             rhs=xhc[kt][:, :nf],
                             start=(kt == 0), stop=(kt == 3))
        nc.scalar.activation(out=g.h12[:, c0:c0 + nf], in_=ps1[:, :nf],
                             func=AF.Identity, bias=g.pw1b_t[:])


def _stage_z(g):
    """z projection, deferred out of the head: re-DMAs x chunks (reusing
    stageA's xin buffers) and runs on the PE/Act slack under scan(cg0)."""
    nc, fp, fps = g.nc, g.fp, g.fps
    for (c0, nf) in MM_CHUNKS:
        xin = [fp.tile([128, 512], BF16, tag=f"xin{t}", bufs=1, name=f"zin{t}") for t in range(2)]
        for t in range(2):
            nc.sync.dma_start(out=xin[t][:, :nf],
                              in_=g.xT_d[t * 128:(t + 1) * 128, c0:c0 + nf])
        for cg in range(2):
            psz = fps.tile([ESH, 512], F32, tag="fps", bufs=2, name="ps_z")
            for kt in range(2):
                nc.tensor.matmul(psz[:, :nf],
                                 lhsT=g.w_in_t[kt][:, E + cg * ESH:E + (cg + 1) * ESH],
                                 rhs=xin[kt][:, :nf],
                                 start=(kt == 0), stop=(kt == 1))
            nc.scalar.activation(out=g.z_sl[cg][:, c0:c0 + nf], in_=psz[:, :nf],
                                 func=AF.Copy)


def _front_conv(g):
    """Depthwise 3x3 on h12 -> accB, in 4 row-bands so each band's ops
    start as soon as the h12 chunks covering its rows (+1 halo) land."""
    nc, fp = g.nc, g.fp
    acc = fp.tile([MID, L], BF16, tag="dwacc")
    acc3 = acc[:].rearrange("p (h w) -> p h w", w=W)
    h3 = g.h12[:].rearrange("p (h w) -> p h w", w=W)
    BAND = 12
    for b0 in range(0, H, BAND):
        b1 = b0 + BAND
        nc.vector.tensor_scalar(out=acc3[:, b0:b1, :],
                                in0=h3[:, b0:b1, :],
                                scalar1=g.dwtap_t[:, 4:5], scalar2=None,
                                op0=OP.mult)
        for ky in range(3):
            for kx in range(3):
                if ky == 1 and kx == 1:
                    continue
                dy, dx = ky - 1, kx - 1
                r0 = max(max(0, -dy), b0)
                r1 = min(H - max(0, dy), b1)
                if r0 >= r1:
                    continue
                w0, w1 = max(0, -dx), W - max(0, dx)
                nc.vector.scalar_tensor_tensor(
                    out=acc3[:, r0:r1, w0:w1],
                    in0=h3[:, r0 + dy:r1 + dy, w0 + dx:w1 + dx],
                    scalar=g.dwtap_t[:, ky * 3 + kx:ky * 3 + kx + 1],
                    in1=acc3[:, r0:r1, w0:w1],
                    op0=OP.mult, op1=OP.add)
        nc.scalar.activation(out=g.accB[:, b0 * W:b1 * W],
                             in_=acc[:, b0 * W:b1 * W], func=AF.Copy)


def _front_stageB(g):
    """pw2+SiLU -> xc chunks -> (xc_sl slice, x_dbl, delta, B/C to DRAM)."""
    nc, fp, fps = g.nc, g.fp, g.fps
    for (c0, nf) in MM_CHUNKS:
        xcc = [fp.tile([128, 512], BF16, tag=f"xcc{m}", bufs=2, name=f"xcc{m}") for m in range(4)]
        for m in range(4):
            ps2 = fps.tile([128, 512], F32, tag="fps", bufs=2, name="ps_pw2")
            nc.tensor.matmul(ps2[:, :nf],
                             lhsT=g.w_pw2_t[:, m * 128:(m + 1) * 128],
                             rhs=g.accB[:, c0:c0 + nf], start=True, stop=True)
            nc.scalar.activation(out=xcc[m][:, :nf], in_=ps2[:, :nf],
                                 func=AF.Silu)
        for cg in range(2):
            nc.sync.dma_start(out=g.xc_sl[cg][:, c0:c0 + nf],
                              in_=xcc[0][cg * ESH:(cg + 1) * ESH, :nf])
        ps3 = fps.tile([64, 512], F32, tag="fps", bufs=2, name="ps_xdbl")
        for kt in range(4):
            nc.tensor.matmul(ps3[:, :nf], lhsT=g.w_xp_t[kt][:],
                             rhs=xcc[kt][:, :nf],
                             start=(kt == 0), stop=(kt == 3))
        nc.scalar.activation(out=g.xdbl[:, c0:c0 + nf], in_=ps3[:, :nf],
                             func=AF.Copy)
        for cg in range(2):
            ps4 = fps.tile([ESH, 512], F32, tag="fps", bufs=2, name="ps_dt")
            nc.tensor.matmul(ps4[:, :nf],
                             lhsT=g.w_dt_t[:, cg * ESH:(cg + 1) * ESH],
                             rhs=g.xdbl[0:R, c0:c0 + nf], start=True, stop=True)
            nc.scalar.activation(out=g.dtlin[cg][:, c0:c0 + nf], in_=ps4[:, :nf],
                                 func=AF.Copy)
    # softplus(v) = ln(1 + exp(v)); |v| < ~10 here so exp cannot overflow.
    # Batched over full L to avoid ACT table reloads inside the chunk loop.
    for cg in range(2):
        nc.scalar.activation(out=g.delta_rep[cg][0:ESH, :], in_=g.dtlin[cg][:],
                             func=AF.Exp, bias=g.spb_t[cg][:])
        nc.scalar.activation(out=g.delta_rep[cg][0:ESH, :],
                             in_=g.delta_rep[cg][0:ESH, :], func=AF.Ln, bias=1.0)
        # duplicate delta rows [0:64] -> [64:128]
        nc.sync.dma_start(out=g.delta_rep[cg][ESH:2 * ESH, :],
                          in_=g.delta_rep[cg][0:ESH, :])
    # B/C rows to DRAM for later partition-broadcast loads
    nc.sync.dma_start(out=g.bsrc[:], in_=g.xdbl[R:R + N, :])
    nc.sync.dma_start(out=g.csrc[:], in_=g.xdbl[R + N:R + 2 * N, :])


def _scan_prep(g, cg):
    """y_acc init (D*u skip), fused-pair dA exps (with zero seam)."""
    nc, sp, wp = g.nc, g.sp, g.wp
    nc.scalar.activation(out=g.y_acc[cg][:], in_=g.xc_sl[cg][:],
                         func=AF.Identity, bias=g.dpb_t[cg][:],
                         scale=g.dp4_t[cg][:])
    for jp in range(NJ // 2):
        # dA for pair (2jp, 2jp+1) fused along free dim; the seam column
        # (fused position L = j-odd's t=0) is zeroed so the recurrence
        # restarts: h = 0*h_prev + dbu = the correct fresh-scan init.
        g.dA[jp] = sp.tile([128, 2 * L], BF16, tag=f"dA{jp}", name=f"dA{jp}")
        for s in range(2):
            nc.scalar.activation(out=g.dA[jp][:, s * L:(s + 1) * L],
                                 in_=g.delta_rep[cg][:], func=AF.Exp,
                                 scale=g.ascale_t[cg][:, 2 * jp + s:2 * jp + s + 1])
        nc.vector.memset(g.dA[jp][:, L:L + 1], 0.0)



def _mk_du(g, cg, k):
    """u_k = perm_k(xc)+dir_k on Act; du = delta*u on DVE; ns-dup DMA.
    Called one direction early so the chain never blocks a k start."""
    nc, wp = g.nc, g.wp
    xc3 = g.xc_sl[cg][:].rearrange("p (h w) -> p h w", w=W)
    xcT = g.xc_sl[cg][:].rearrange("p (h w) -> p w h", w=W)
    usrc = [xc3, xc3[:, ::-1, ::-1], xcT, xcT[:, ::-1, ::-1]][k]
    u_tmp = wp.tile([ESH, L], BF16, tag="u_tmp", bufs=2)
    u3 = u_tmp[:].rearrange("p (a c) -> p a c", c=W)
    nc.scalar.activation(out=u3, in_=usrc, func=AF.Identity,
                         bias=g.dire_t[cg][:, k:k + 1])
    du = wp.tile([128, L], BF16, tag="du", bufs=2)
    nc.vector.tensor_tensor(out=du[0:ESH, :], in0=g.delta_rep[cg][0:ESH, :],
                            in1=u_tmp[:], op=OP.mult)
    nc.sync.dma_start(out=du[ESH:2 * ESH, :], in_=du[0:ESH, :])
    return du


def _scan_k(g, cg, k, du, pending=None, prefetch=None):
    """One direction's 8 scan tiles (scans fused in j-pairs).
    `prefetch` (emitted mid-direction) builds the next direction's du.
    Returns (drain closure, prefetched du)."""
    nc, wp, yps = g.nc, g.wp, g.yps
    du_next = None
    ypsum = [yps.tile([ESH, HALF], F32, tag=f"yps{h}", name=f"yps{h}") for h in range(2)]
    for jp in range(NJ // 2):
        dbu = wp.tile([128, 2 * L], BF16, tag="workA", bufs=1)
        for s in range(2):
            B_t = wp.tile([128, L], BF16, tag="B_t", bufs=2)
            C_s = [None, None]
            for ns in range(2):
                row = 4 * jp + 2 * s + ns
                nc.sync.dma_start(
                    out=B_t[ns * ESH:(ns + 1) * ESH, :],
                    in_=g.bsrc[row:row + 1, :].to_broadcast((ESH, L)))
            nc.vector.tensor_tensor(out=dbu[:, s * L:(s + 1) * L],
                                    in0=du[:], in1=B_t[:], op=OP.mult)
        h_t = wp.tile([128, 2 * L], BF16, tag="workH", bufs=1)
        nc.vector.tensor_tensor_scan(out=h_t[:], data0=g.dA[jp][:],
                                     data1=dbu[:], initial=0.0,
                                     op0=OP.mult, op1=OP.add)
        if jp == 0 and pending is not None:
            # previous direction's PSUM drain: emitted here (before this
            # direction's first start=True sel-matmul touches ypsum) but
            # after ~12us of fused dbu+scan, so it no longer stalls the
            # DVE on the PE's last sel-matmul of the previous direction.
            pending()
        if jp == 1 and prefetch is not None:
            du_next = prefetch()
        for s in range(2):
            C_t = wp.tile([128, L], BF16, tag="C_t", bufs=2)
            for ns in range(2):
                row = 4 * jp + 2 * s + ns
                nc.sync.dma_start(
                    out=C_t[ns * ESH:(ns + 1) * ESH, :],
                    in_=g.csrc[row:row + 1, :].to_broadcast((ESH, L)))
            hc = wp.tile([128, L], BF16, tag="workB", bufs=2)
            nc.vector.tensor_tensor(out=hc[:], in0=h_t[:, s * L:(s + 1) * L],
                                    in1=C_t[:], op=OP.mult)
            for hh in range(2):
                for (c0, nf) in MM_CHUNKS_HALF:
                    nc.tensor.matmul(
                        ypsum[hh][:, c0:c0 + nf],
                        lhsT=g.sel_t[:],
                        rhs=hc[:, hh * HALF + c0:hh * HALF + c0 + nf],
                        start=(jp == 0 and s == 0),
                        stop=(jp == NJ // 2 - 1 and s == 1))
    def drain():
        # accumulate un-permuted ys_k into y_acc
        for hh in range(2):
            pv = ypsum[hh][:]
            if k == 0:
                dst = g.y_acc[cg][:, hh * HALF:(hh + 1) * HALF]
                srcv = pv
            elif k == 1:
                dst = g.y_acc[cg][:, (1 - hh) * HALF:(2 - hh) * HALF]
                srcv = pv[:, ::-1]
            elif k == 2:
                # ys[i], i=a*48+b_ -> l = b_*48+a ; half hh: a in [24hh,...)
                dst = g.y_acc[cg][:].rearrange("p (bb a) -> p bb a", a=W)[
                    :, :, 24 * hh:24 * hh + 24]
                srcv = pv.rearrange("p (a bb) -> p bb a", bb=W)
            else:
                dst = g.y_acc[cg][:].rearrange("p (bb a) -> p bb a", a=W)[
                    :, :, 24 * (1 - hh):24 * (1 - hh) + 24]
                srcv = pv.rearrange("p (a bb) -> p bb a", bb=W)[:, ::-1, ::-1]
            nc.vector.tensor_tensor(out=dst, in0=srcv, in1=dst, op=OP.add)
    return drain, du_next


def _finish_yv(g, cg):
    """yv[cg] = y_acc * silu(z) for this channel group."""
    nc, wp = g.nc, g.wp
    sz = wp.tile([ESH, L], BF16, tag="u_tmp", bufs=2)
    nc.scalar.activation(out=sz[:], in_=g.z_sl[cg][:], func=AF.Silu)
    g.yv[cg] = g.fp.tile([ESH, L], BF16, tag=f"dtlin{cg}", name=f"yv{cg}")
    nc.vector.tensor_tensor(out=g.yv[cg][:], in0=g.y_acc[cg][:], in1=sz[:],
                            op=OP.mult)


def _out_proj(g):
    """out_partial = sum_cg W_out[cg]^T @ yv[cg] (PSUM-accumulated)."""
    nc, wp, fps = g.nc, g.wp, g.fps
    for m in range(2):
        for (c0, nf) in MM_CHUNKS:
            po = fps.tile([128, 512], F32, tag="fps", bufs=2, name="ps_out")
            for cg in range(2):
                nc.tensor.matmul(po[:, :nf],
                                 lhsT=g.w_out_t[cg][:, m * 128:(m + 1) * 128],
                                 rhs=g.yv[cg][:, c0:c0 + nf],
                                 start=(cg == 0), stop=(cg == 1))
            osb = wp.tile([128, 512], F32, tag="osb", bufs=1)
            nc.scalar.activation(out=osb[:, :nf], in_=po[:, :nf], func=AF.Copy)
            nc.sync.dma_start(out=g.out_d[m * 128:(m + 1) * 128, c0:c0 + nf],
                              in_=osb[:, :nf])


def _r32r(a):
    """Round fp32 -> fp32r (TF32-like, 10 explicit mantissa bits)."""
    b = np.ascontiguousarray(a, np.float32).view(np.uint32)
    return (((b.astype(np.uint64) + 0x1000) & 0xFFFFE000)
            .astype(np.uint32).view(np.float32))


def _bf16(a):
    return np.ascontiguousarray(np.asarray(a, np.float32)).astype(
        ml_dtypes.bfloat16)


def _host_prep(inputs):
    x = np.asarray(inputs["x"], np.float32)
    W_pos = np.asarray(inputs["W_pos"], np.float32)
    b_pos = np.asarray(inputs["b_pos"], np.float32)
    W_in = np.asarray(inputs["W_in"], np.float32)
    pw1_w = np.asarray(inputs["pw1_w"], np.float32)
    pw1_b = np.asarray(inputs["pw1_b"], np.float32)
    dw_w = np.asarray(inputs["dw_w"], np.float32)
    pw2_w = np.asarray(inputs["pw2_w"], np.float32)
    W_xproj = np.asarray(inputs["W_xproj"], np.float32)
    W_dt = np.asarray(inputs["W_dt"], np.float32)
    b_dt = np.asarray(inputs["b_dt"], np.float32)
    A_log = np.asarray(inputs["A_log"], np.float32)
    Dp = np.asarray(inputs["Dp"], np.float32)
    dir_emb = np.asarray(inputs["dir_emb"], np.float32)
    W_out = np.asarray(inputs["W_out"], np.float32)

    gy, gx = np.meshgrid(np.arange(H, dtype=np.float32),
                         np.arange(W, dtype=np.float32), indexing="ij")
    coords = np.stack([gy, gx], -1) / (H - 1) * 2 - 1
    pos = (coords.reshape(L, 2) @ W_pos + b_pos).astype(np.float32)

    common = {
        "w_pw1": _bf16(pw1_w.reshape(MID, E).T),
        "pw1b": np.ascontiguousarray(pw1_b.reshape(MID, 1)),
        "dwtap": np.ascontiguousarray(dw_w.reshape(MID, 9)),
    }
    w_pw2_base = pw2_w.reshape(E, MID).T  # (MID, E)
    A = -np.exp(A_log)  # (E, N)
    xp = (x + pos[None]).transpose(0, 2, 1)  # (B, Dm, L)

    sel = np.zeros((2 * ESH, ESH), np.float32)
    for p in range(2 * ESH):
        sel[p, p % ESH] = 1.0
    sel = sel.astype(ml_dtypes.bfloat16)

    in_maps = []
    for c in range(NCORES):
        bcr = c // 4               # this core's batch
        e0 = (c % 4) * 2 * ESH     # this core's 128-channel slice
        sl = slice(e0, e0 + 2 * ESH)
        ascale = np.empty((2, 2 * ESH, NJ), np.float32)
        for cg in range(2):
            A_cg = A[e0 + cg * ESH:e0 + (cg + 1) * ESH]  # (64, 16)
            for p in range(2 * ESH):
                for j in range(NJ):
                    ascale[cg, p, j] = A_cg[p % ESH, 2 * j + p // ESH]
        m = dict(common)
        m["xT"] = _bf16(xp[bcr])
        # channel permutation putting this core's slice at rows [0:128]
        perm = np.concatenate([np.arange(e0, e0 + 2 * ESH),
                               np.arange(0, e0),
                               np.arange(e0 + 2 * ESH, E)])
        m["w_pw2"] = _bf16(w_pw2_base[:, perm])
        m["w_xp"] = _bf16(np.concatenate(
            [W_xproj[perm, :], np.zeros((E, 64 - (R + 2 * N)), np.float32)],
            axis=1))
        m["w_in"] = _bf16(
            np.concatenate([W_in[:, :E], W_in[:, E + e0:E + e0 + 2 * ESH]],
                           axis=1))
        m["w_dt"] = _bf16(W_dt[:, sl])
        m["spb"] = np.ascontiguousarray(
            (2.0 * b_dt[sl]).reshape(2, ESH, 1))
        m["ascale"] = ascale
        m["dire"] = np.ascontiguousarray(
            dir_emb[:, sl].T.reshape(2, ESH, 4))
        m["dp4"] = np.ascontiguousarray((4.0 * Dp[sl]).reshape(2, ESH, 1))
        m["dpb"] = np.ascontiguousarray(
            (Dp[sl] * dir_emb[:, sl].sum(0)).reshape(2, ESH, 1))
        m["w_out"] = _bf16(W_out[sl, :].reshape(2, ESH, DM))
        m["sel"] = sel
        in_maps.append(m)
    return in_maps


_PROGRAM = None
_LAST_RESULTS = None
_LAST_INSTS = None


def _get_program():
    global _PROGRAM
    if _PROGRAM is None:
        _PROGRAM = build_program()
    return _PROGRAM


def kernel(**inputs):
    global _LAST_EXEC_NS, _LAST_RESULTS
    assert int(inputs["H"]) == H and int(inputs["W"]) == W
    in_maps = _host_prep(inputs)
    if TRACE:
        _install_profile_shim()
    res = run_bass_kernel_spmd(_get_program(), in_maps,
                               list(range(NCORES)), trace=TRACE)
    _LAST_EXEC_NS = res.exec_time_ns
    _LAST_RESULTS = res.results
    global _LAST_INSTS
    _LAST_INSTS = res.instructions_and_trace
    out = np.zeros((B, DM, L), np.float32)
    for c, r in enumerate(res.results):
        out[c // 4] += np.asarray(r["out"], np.float32)
    return np.ascontiguousarray(out.transpose(0, 2, 1))


# revision 38
# speedup vs baseline: 1.0129x; 1.0129x over previous
"""Trainium2 Bass kernel for the LIDAR2D 4-direction selective-scan block.

Sharding: (batch, d_inner/4). Core c handles batch c//4 and a 128-channel
slice of d_inner (E=512): the host passes x[b] and channel-sliced scan
params per core, so the program stays SPMD-identical. Each core computes
the full-E front for its batch once, then scans its 128 channels as two
64-channel groups (cg) x 16 states x 4 directions, and emits a partial
out-projection (Dm, L). The host sums 4 partials per batch.

Scan layout per core: tiles of [128 partitions = (nsub in {0,1}) x (64
channels), free = L] — 8 tiles j=0..7 cover states n = 2j + nsub. The
recurrence h_t = exp(A*delta_t)*h_{t-1} + delta_t*B_t*u_t runs in a single
DVE tensor_tensor_scan per tile (2 cyc/elem — the hardware floor and the
kernel's critical path). y_t = C_t . h_t is a PE matmul with a 0/1
selection matrix contracting the two nsub rows per channel, accumulated
over j in PSUM. Direction permutations (reverse / spatial transpose) are
pure access-pattern tricks on reads/writes.

Perf structure: the front runs in bf16 (1 PE pass/col) with per-512-chunk
transient tiles. With one batch per core there is no second front to
overlap; the DVE stream is the 64 scans plus their elementwise feeds, and
the PE/Act fronts run only in the ~100us head.
"""

import os
import sys

for _p in ("/opt/trn_rl_repo", os.path.expanduser("~/.axon_site/_ro/trn_rl_repo")):
    if os.path.isdir(_p) and _p not in sys.path:
        sys.path.insert(0, _p)

import numpy as np
import ml_dtypes

import concourse.bass as bass
import concourse.bacc as bacc
import concourse.mybir as mybir
from concourse.tile import TileContext
from concourse.bass_utils import run_bass_kernel_spmd

F32 = mybir.dt.float32
F32R = mybir.dt.float32r
BF16 = mybir.dt.bfloat16
AF = mybir.ActivationFunctionType
OP = mybir.AluOpType

# Problem shape (hardcoded per the harness contract).
B, L, DM, E, N, R, MID, H, W = 2, 2304, 256, 512, 16, 16, 32, 48, 48
NCORES = 8
ESH = E // NCORES          # 64 channels per core
NJ = N // 2                # 8 scan tiles per (b, k); rows = (nsub, e_loc)
HALF = L // 2              # 1152, for PSUM-sized y accumulation

TRACE = bool(os.environ.get("KERNEL_TRACE"))
_LAST_EXEC_NS = None


def _install_profile_shim():
    """Make run_bass_kernel_spmd(trace=True) work in this container:
    register the NTFF hook (antenv.axon_hooks is absent here) and stub
    the S3 artifact upload."""
    import types
    try:
        from antenv.axon_hooks import get_axon_ntff_profile_hook  # noqa: F401
    except ImportError:
        import antenv
        mod = types.ModuleType("antenv.axon_hooks")
        mod._HOOK = None
        mod.set_axon_ntff_profile_hook = lambda h: setattr(mod, "_HOOK", h)
        mod.get_axon_ntff_profile_hook = lambda: mod._HOOK
        sys.modules["antenv.axon_hooks"] = mod
        antenv.axon_hooks = mod
        try:
            from trn_agent_boot.trn_boot import _ntff_profile_via_ctypes
            hook = _ntff_profile_via_ctypes("/opt/axon/libaxon_pjrt.so")
            if hook is not None:
                mod._HOOK = hook
        except Exception as e:  # pragma: no cover
            print(f"profile shim: hook install failed: {e}")
    import concourse.bass_utils as bu
    bu.upload_artifacts = lambda tmpdir: f"file://{tmpdir}"


def _chunks(total, step):
    out = []
    c0 = 0
    while c0 < total:
        out.append((c0, min(step, total - c0)))
        c0 += step
    return out


MM_CHUNKS = _chunks(L, 512)          # matmul free-dim chunks over full L
MM_CHUNKS_HALF = _chunks(HALF, 512)  # chunks within a 1152 half


class Env:
    pass


def build_program():
    nc = bacc.Bacc()
    g = Env()
    g.nc = nc

    # ---- DRAM parameters (same shapes on every core; values differ) ----
    g.xT_d = nc.declare_dram_parameter("xT", [DM, L], BF16, isOutput=False)
    g.w_in_d = nc.declare_dram_parameter("w_in", [DM, E + 2 * ESH], BF16, isOutput=False)
    g.w_pw1_d = nc.declare_dram_parameter("w_pw1", [E, MID], BF16, isOutput=False)
    g.pw1b_d = nc.declare_dram_parameter("pw1b", [MID, 1], F32, isOutput=False)
    g.dwtap_d = nc.declare_dram_parameter("dwtap", [MID, 9], F32, isOutput=False)
    g.w_pw2_d = nc.declare_dram_parameter("w_pw2", [MID, E], BF16, isOutput=False)
    g.w_xp_d = nc.declare_dram_parameter("w_xp", [E, 64], BF16, isOutput=False)
    g.w_dt_d = nc.declare_dram_parameter("w_dt", [R, 2 * ESH], BF16, isOutput=False)
    g.spb_d = nc.declare_dram_parameter("spb", [2, ESH, 1], F32, isOutput=False)
    g.ascale_d = nc.declare_dram_parameter("ascale", [2, 2 * ESH, NJ], F32, isOutput=False)
    g.dire_d = nc.declare_dram_parameter("dire", [2, ESH, 4], F32, isOutput=False)
    g.dp4_d = nc.declare_dram_parameter("dp4", [2, ESH, 1], F32, isOutput=False)
    g.dpb_d = nc.declare_dram_parameter("dpb", [2, ESH, 1], F32, isOutput=False)
    g.w_out_d = nc.declare_dram_parameter("w_out", [2, ESH, DM], BF16, isOutput=False)
    g.sel_d = nc.declare_dram_parameter("sel", [2 * ESH, ESH], BF16, isOutput=False)
    g.out_d = nc.declare_dram_parameter("out", [DM, L], F32, isOutput=True)

    with TileContext(nc) as tc:
        g.tc = tc
        with tc.tile_pool(name="const", bufs=1) as cp, \
             tc.tile_pool(name="persist", bufs=1) as pp, \
             tc.tile_pool(name="front", bufs=1) as fp, \
             tc.tile_pool(name="scan", bufs=1) as sp, \
             tc.tile_pool(name="work", bufs=2) as wp, \
             tc.tile_pool(name="fpsum", bufs=2, space="PSUM") as fps, \
             tc.tile_pool(name="ypsum", bufs=1, space="PSUM") as yps, \
             tc.tile_pool(name="bounce", bufs=1, space="DRAM") as bp:
            g.cp, g.pp, g.fp, g.sp, g.wp, g.fps, g.yps, g.bp = \
                cp, pp, fp, sp, wp, fps, yps, bp
            _load_consts(g)

            # persistent per-b products of the front
            g.xc_sl = [pp.tile([ESH, L], BF16, tag=f"xc_sl{b}", name=f"xc_sl{b}") for b in range(B)]
            g.delta_rep = [pp.tile([128, L], F32, tag=f"drep{b}", name=f"drep{b}") for b in range(B)]
            g.z_sl = [pp.tile([ESH, L], BF16, tag=f"z{b}", name=f"z{b}") for b in range(B)]
            g.y_acc = [pp.tile([ESH, L], F32, tag=f"yacc{b}", name=f"yacc{b}") for b in range(B)]
            g.bsrc = bp.tile([N, L], BF16, tag="bsrc", name="bsrc")
            g.csrc = bp.tile([N, L], BF16, tag="csrc", name="csrc")
            # h12/acc full-L per-b (conv needs the whole plane)
            g.h12 = fp.tile([MID, L], BF16, tag="h12", name="h12")
            g.accB = fp.tile([MID, L], BF16, tag="accB", name="accB")
            g.xdbl = fp.tile([64, L], BF16, tag="xdbl", name="xdbl")
            g.dtlin = [fp.tile([ESH, L], BF16, tag=f"dtlin{c}", name=f"dtlin{c}") for c in range(2)]
            g.dA = [None] * NJ
            g.yv = [None, None]

            # ---- emission schedule (one batch per core, two cg groups) ----
            _front_stageA(g)
            _front_conv(g)
            _front_stageB(g)
            _scan_prep(g, 0)
            pend = _scan_k(g, 0, 0)
            _stage_z(g)
            for k in range(1, 4):
                pend = _scan_k(g, 0, k, pending=pend)
            _scan_prep(g, 1)
            pend()
            _finish_yv(g, 0)
            pend = None
            for k in range(4):
                pend = _scan_k(g, 1, k, pending=pend)
            pend()
            _finish_yv(g, 1)
            _out_proj(g)

    nc.finalize()
    return nc


def _load_consts(g):
    nc, cp = g.nc, g.cp
    g.w_in_t = [cp.tile([128, E + 2 * ESH], BF16, tag=f"w_in{t}", name=f"w_in{t}") for t in range(2)]
    for t in range(2):
        nc.sync.dma_start(out=g.w_in_t[t][:], in_=g.w_in_d[t * 128:(t + 1) * 128, :])
    g.w_pw1_t = [cp.tile([128, MID], BF16, tag=f"w_pw1{t}", name=f"w_pw1{t}") for t in range(4)]
    for t in range(4):
        nc.sync.dma_start(out=g.w_pw1_t[t][:], in_=g.w_pw1_d[t * 128:(t + 1) * 128, :])
    g.pw1b_t = cp.tile([MID, 1], F32, tag="pw1b", name="pw1b")
    nc.sync.dma_start(out=g.pw1b_t[:], in_=g.pw1b_d[:])
    g.dwtap_t = cp.tile([MID, 9], F32, tag="dwtap", name="dwtap")
    nc.sync.dma_start(out=g.dwtap_t[:], in_=g.dwtap_d[:])
    g.w_pw2_t = cp.tile([MID, E], BF16, tag="w_pw2", name="w_pw2")
    nc.sync.dma_start(out=g.w_pw2_t[:], in_=g.w_pw2_d[:])
    g.w_xp_t = [cp.tile([128, 64], BF16, tag=f"w_xp{t}", name=f"w_xp{t}") for t in range(4)]
    for t in range(4):
        nc.sync.dma_start(out=g.w_xp_t[t][:], in_=g.w_xp_d[t * 128:(t + 1) * 128, :])
    g.w_dt_t = cp.tile([R, 2 * ESH], BF16, tag="w_dt", name="w_dt")
    nc.sync.dma_start(out=g.w_dt_t[:], in_=g.w_dt_d[:])
    g.spb_t = [cp.tile([ESH, 1], F32, tag=f"spb{c}", name=f"spb{c}") for c in range(2)]
    g.ascale_t = [cp.tile([2 * ESH, NJ], F32, tag=f"ascale{c}", name=f"ascale{c}") for c in range(2)]
    g.dire_t = [cp.tile([ESH, 4], F32, tag=f"dire{c}", name=f"dire{c}") for c in range(2)]
    g.dp4_t = [cp.tile([ESH, 1], F32, tag=f"dp4{c}", name=f"dp4{c}") for c in range(2)]
    g.dpb_t = [cp.tile([ESH, 1], F32, tag=f"dpb{c}", name=f"dpb{c}") for c in range(2)]
    g.w_out_t = [cp.tile([ESH, DM], BF16, tag=f"w_out{c}", name=f"w_out{c}") for c in range(2)]
    for c in range(2):
        nc.sync.dma_start(out=g.spb_t[c][:], in_=g.spb_d[c])
        nc.sync.dma_start(out=g.ascale_t[c][:], in_=g.ascale_d[c])
        nc.sync.dma_start(out=g.dire_t[c][:], in_=g.dire_d[c])
        nc.sync.dma_start(out=g.dp4_t[c][:], in_=g.dp4_d[c])
        nc.sync.dma_start(out=g.dpb_t[c][:], in_=g.dpb_d[c])
        nc.sync.dma_start(out=g.w_out_t[c][:], in_=g.w_out_d[c])
    g.sel_t = cp.tile([2 * ESH, ESH], BF16, tag="sel", name="sel")
    nc.sync.dma_start(out=g.sel_t[:], in_=g.sel_d[:])


def _front_stageA(g):
    """x+pos -> xz -> (z slices, xh) -> pw1 -> h12 (full [MID, L])."""
    nc, fp, fps = g.nc, g.fp, g.fps
    for (c0, nf) in MM_CHUNKS:
        xin = [fp.tile([128, 512], BF16, tag=f"xin{t}", bufs=2, name=f"xin{t}") for t in range(2)]
        for t in range(2):
            nc.sync.dma_start(out=xin[t][:, :nf],
                              in_=g.xT_d[t * 128:(t + 1) * 128, c0:c0 + nf])
        xhc = [fp.tile([128, 512], BF16, tag=f"xhc{m}", bufs=2, name=f"xhc{m}") for m in range(4)]
        for m in range(4):
            ps = fps.tile([128, 512], F32, tag="fps", bufs=2, name="ps_xz")
            for kt in range(2):
                nc.tensor.matmul(ps[:, :nf],
                                 lhsT=g.w_in_t[kt][:, m * 128:(m + 1) * 128],
                                 rhs=xin[kt][:, :nf],
                                 start=(kt == 0), stop=(kt == 1))
            nc.scalar.activation(out=xhc[m][:, :nf], in_=ps[:, :nf], func=AF.Copy)
        ps1 = fps.tile([MID, 512], F32, tag="fps", bufs=2, name="ps_pw1")
        for kt in range(4):
            nc.tensor.matmul(ps1[:, :nf], lhsT=g.w_pw1_t[kt][:],
                             rhs=xhc[kt][:, :nf],
                             start=(kt == 0), stop=(kt == 3))
        nc.scalar.activation(out=g.h12[:, c0:c0 + nf], in_=ps1[:, :nf],
                             func=AF.Identity, bias=g.pw1b_t[:])


def _stage_z(g):
    """z projection, deferred out of the head: re-DMAs x chunks (reusing
    stageA's xin buffers) and runs on the PE/Act slack under scan(cg0)."""
    nc, fp, fps = g.nc, g.fp, g.fps
    for (c0, nf) in MM_CHUNKS:
        xin = [fp.tile([128, 512], BF16, tag=f"xin{t}", bufs=2, name=f"zin{t}") for t in range(2)]
        for t in range(2):
            nc.sync.dma_start(out=xin[t][:, :nf],
                              in_=g.xT_d[t * 128:(t + 1) * 128, c0:c0 + nf])
        for cg in range(2):
            psz = fps.tile([ESH, 512], F32, tag="fps", bufs=2, name="ps_z")
            for kt in range(2):
                nc.tensor.matmul(psz[:, :nf],
                                 lhsT=g.w_in_t[kt][:, E + cg * ESH:E + (cg + 1) * ESH],
                                 rhs=xin[kt][:, :nf],
                                 start=(kt == 0), stop=(kt == 1))
            nc.scalar.activation(out=g.z_sl[cg][:, c0:c0 + nf], in_=psz[:, :nf],
                                 func=AF.Copy)


def _front_conv(g):
    """Depthwise 3x3 on h12 -> accB, in 4 row-bands so each band's ops
    start as soon as the h12 chunks covering its rows (+1 halo) land."""
    nc, fp = g.nc, g.fp
    acc = fp.tile([MID, L], BF16, tag="dwacc")
    acc3 = acc[:].rearrange("p (h w) -> p h w", w=W)
    h3 = g.h12[:].rearrange("p (h w) -> p h w", w=W)
    BAND = 12
    for b0 in range(0, H, BAND):
        b1 = b0 + BAND
        nc.vector.tensor_scalar(out=acc3[:, b0:b1, :],
                                in0=h3[:, b0:b1, :],
                                scalar1=g.dwtap_t[:, 4:5], scalar2=None,
                                op0=OP.mult)
        for ky in range(3):
            for kx in range(3):
                if ky == 1 and kx == 1:
                    continue
                dy, dx = ky - 1, kx - 1
                r0 = max(max(0, -dy), b0)
                r1 = min(H - max(0, dy), b1)
                if r0 >= r1:
                    continue
                w0, w1 = max(0, -dx), W - max(0, dx)
                nc.vector.scalar_tensor_tensor(
                    out=acc3[:, r0:r1, w0:w1],
                    in0=h3[:, r0 + dy:r1 + dy, w0 + dx:w1 + dx],
                    scalar=g.dwtap_t[:, ky * 3 + kx:ky * 3 + kx + 1],
                    in1=acc3[:, r0:r1, w0:w1],
                    op0=OP.mult, op1=OP.add)
        nc.scalar.activation(out=g.accB[:, b0 * W:b1 * W],
                             in_=acc[:, b0 * W:b1 * W], func=AF.Copy)


def _front_stageB(g):
    """pw2+SiLU -> xc chunks -> (xc_sl slice, x_dbl, delta, B/C to DRAM)."""
    nc, fp, fps = g.nc, g.fp, g.fps
    for (c0, nf) in MM_CHUNKS:
        xcc = [fp.tile([128, 512], BF16, tag=f"xcc{m}", bufs=2, name=f"xcc{m}") for m in range(4)]
        for m in range(4):
            ps2 = fps.tile([128, 512], F32, tag="fps", bufs=2, name="ps_pw2")
            nc.tensor.matmul(ps2[:, :nf],
                             lhsT=g.w_pw2_t[:, m * 128:(m + 1) * 128],
                             rhs=g.accB[:, c0:c0 + nf], start=True, stop=True)
            nc.scalar.activation(out=xcc[m][:, :nf], in_=ps2[:, :nf],
                                 func=AF.Silu)
        for cg in range(2):
            nc.sync.dma_start(out=g.xc_sl[cg][:, c0:c0 + nf],
                              in_=xcc[0][cg * ESH:(cg + 1) * ESH, :nf])
        ps3 = fps.tile([64, 512], F32, tag="fps", bufs=2, name="ps_xdbl")
        for kt in range(4):
            nc.tensor.matmul(ps3[:, :nf], lhsT=g.w_xp_t[kt][:],
                             rhs=xcc[kt][:, :nf],
                             start=(kt == 0), stop=(kt == 3))
        nc.scalar.activation(out=g.xdbl[:, c0:c0 + nf], in_=ps3[:, :nf],
                             func=AF.Copy)
        for cg in range(2):
            ps4 = fps.tile([ESH, 512], F32, tag="fps", bufs=2, name="ps_dt")
            nc.tensor.matmul(ps4[:, :nf],
                             lhsT=g.w_dt_t[:, cg * ESH:(cg + 1) * ESH],
                             rhs=g.xdbl[0:R, c0:c0 + nf], start=True, stop=True)
            nc.scalar.activation(out=g.dtlin[cg][:, c0:c0 + nf], in_=ps4[:, :nf],
                                 func=AF.Copy)
    # softplus(v) = ln(1 + exp(v)); |v| < ~10 here so exp cannot overflow.
    # Batched over full L to avoid ACT table reloads inside the chunk loop.
    for cg in range(2):
        nc.scalar.activation(out=g.delta_rep[cg][0:ESH, :], in_=g.dtlin[cg][:],
                             func=AF.Exp, bias=g.spb_t[cg][:])
        nc.scalar.activation(out=g.delta_rep[cg][0:ESH, :],
                             in_=g.delta_rep[cg][0:ESH, :], func=AF.Ln, bias=1.0)
        # duplicate delta rows [0:64] -> [64:128]
        nc.sync.dma_start(out=g.delta_rep[cg][ESH:2 * ESH, :],
                          in_=g.delta_rep[cg][0:ESH, :])
    # B/C rows to DRAM for later partition-broadcast loads
    nc.sync.dma_start(out=g.bsrc[:], in_=g.xdbl[R:R + N, :])
    nc.sync.dma_start(out=g.csrc[:], in_=g.xdbl[R + N:R + 2 * N, :])


def _scan_prep(g, cg):
    """y_acc init (D*u skip), fused-pair dA exps (with zero seam)."""
    nc, sp, wp = g.nc, g.sp, g.wp
    nc.scalar.activation(out=g.y_acc[cg][:], in_=g.xc_sl[cg][:],
                         func=AF.Identity, bias=g.dpb_t[cg][:],
                         scale=g.dp4_t[cg][:])
    for jp in range(NJ // 2):
        # dA for pair (2jp, 2jp+1) fused along free dim; the seam column
        # (fused position L = j-odd's t=0) is zeroed so the recurrence
        # restarts: h = 0*h_prev + dbu = the correct fresh-scan init.
        g.dA[jp] = sp.tile([128, 2 * L], BF16, tag=f"dA{jp}", name=f"dA{jp}")
        for s in range(2):
            nc.scalar.activation(out=g.dA[jp][:, s * L:(s + 1) * L],
                                 in_=g.delta_rep[cg][:], func=AF.Exp,
                                 scale=g.ascale_t[cg][:, 2 * jp + s:2 * jp + s + 1])
        nc.vector.memset(g.dA[jp][:, L:L + 1], 0.0)



def _scan_k(g, cg, k, pending=None):
    """One direction's du + 8 scan tiles (scans fused in j-pairs).
    Returns a drain closure (ypsum -> y_acc) emitted inside the next
    direction's first-pair window."""
    nc, wp, yps = g.nc, g.wp, g.yps
    xc3 = g.xc_sl[cg][:].rearrange("p (h w) -> p h w", w=W)
    xcT = g.xc_sl[cg][:].rearrange("p (h w) -> p w h", w=W)
    usrc = [xc3, xc3[:, ::-1, ::-1], xcT, xcT[:, ::-1, ::-1]][k]
    u_tmp = wp.tile([ESH, L], BF16, tag="u_tmp", bufs=1)
    u3 = u_tmp[:].rearrange("p (a c) -> p a c", c=W)
    nc.scalar.activation(out=u3, in_=usrc, func=AF.Identity,
                         bias=g.dire_t[cg][:, k:k + 1])
    du = wp.tile([128, L], BF16, tag="du", bufs=2)
    nc.vector.tensor_tensor(out=du[0:ESH, :], in0=g.delta_rep[cg][0:ESH, :],
                            in1=u_tmp[:], op=OP.mult)
    nc.sync.dma_start(out=du[ESH:2 * ESH, :], in_=du[0:ESH, :])
    ypsum = [yps.tile([ESH, HALF], F32, tag=f"yps{h}", name=f"yps{h}") for h in range(2)]
    for jp in range(NJ // 2):
        dbu = wp.tile([128, 2 * L], BF16, tag="workA", bufs=1)
        for s in range(2):
            B_t = wp.tile([128, L], BF16, tag="B_t", bufs=2)
            C_s = [None, None]
            for ns in range(2):
                row = 4 * jp + 2 * s + ns
                nc.sync.dma_start(
                    out=B_t[ns * ESH:(ns + 1) * ESH, :],
                    in_=g.bsrc[row:row + 1, :].to_broadcast((ESH, L)))
            nc.vector.tensor_tensor(out=dbu[:, s * L:(s + 1) * L],
                                    in0=du[:], in1=B_t[:], op=OP.mult)
        h_t = wp.tile([128, 2 * L], BF16, tag="workH", bufs=1)
        nc.vector.tensor_tensor_scan(out=h_t[:], data0=g.dA[jp][:],
                                     data1=dbu[:], initial=0.0,
                                     op0=OP.mult, op1=OP.add)
        if jp == 0 and pending is not None:
            # previous direction's PSUM drain: emitted here (before this
            # direction's first start=True sel-matmul touches ypsum) but
            # after ~12us of fused dbu+scan, so it no longer stalls the
            # DVE on the PE's last sel-matmul of the previous direction.
            pending()
        for s in range(2):
            C_t = wp.tile([128, L], BF16, tag="C_t", bufs=2)
            for ns in range(2):
                row = 4 * jp + 2 * s + ns
                nc.sync.dma_start(
                    out=C_t[ns * ESH:(ns + 1) * ESH, :],
                    in_=g.csrc[row:row + 1, :].to_broadcast((ESH, L)))
            hc = wp.tile([128, L], BF16, tag="workB", bufs=2)
            nc.vector.tensor_tensor(out=hc[:], in0=h_t[:, s * L:(s + 1) * L],
                                    in1=C_t[:], op=OP.mult)
            for hh in range(2):
                for (c0, nf) in MM_CHUNKS_HALF:
                    nc.tensor.matmul(
                        ypsum[hh][:, c0:c0 + nf],
                        lhsT=g.sel_t[:],
                        rhs=hc[:, hh * HALF + c0:hh * HALF + c0 + nf],
                        start=(jp == 0 and s == 0),
                        stop=(jp == NJ // 2 - 1 and s == 1))
    def drain():
        # accumulate un-permuted ys_k into y_acc
        for hh in range(2):
            pv = ypsum[hh][:]
            if k == 0:
                dst = g.y_acc[cg][:, hh * HALF:(hh + 1) * HALF]
                srcv = pv
            elif k == 1:
                dst = g.y_acc[cg][:, (1 - hh) * HALF:(2 - hh) * HALF]
                srcv = pv[:, ::-1]
            elif k == 2:
                # ys[i], i=a*48+b_ -> l = b_*48+a ; half hh: a in [24hh,...)
                dst = g.y_acc[cg][:].rearrange("p (bb a) -> p bb a", a=W)[
                    :, :, 24 * hh:24 * hh + 24]
                srcv = pv.rearrange("p (a bb) -> p bb a", bb=W)
            else:
                dst = g.y_acc[cg][:].rearrange("p (bb a) -> p bb a", a=W)[
                    :, :, 24 * (1 - hh):24 * (1 - hh) + 24]
                srcv = pv.rearrange("p (a bb) -> p bb a", bb=W)[:, ::-1, ::-1]
            nc.vector.tensor_tensor(out=dst, in0=srcv, in1=dst, op=OP.add)
    return drain


def _finish_yv(g, cg):
    """yv[cg] = y_acc * silu(z) for this channel group."""
    nc, wp = g.nc, g.wp
    sz = wp.tile([ESH, L], BF16, tag="u_tmp", bufs=1)
    nc.scalar.activation(out=sz[:], in_=g.z_sl[cg][:], func=AF.Silu)
    g.yv[cg] = g.fp.tile([ESH, L], BF16, tag=f"dtlin{cg}", name=f"yv{cg}")
    nc.vector.tensor_tensor(out=g.yv[cg][:], in0=g.y_acc[cg][:], in1=sz[:],
                            op=OP.mult)


def _out_proj(g):
    """out_partial = sum_cg W_out[cg]^T @ yv[cg] (PSUM-accumulated)."""
    nc, wp, fps = g.nc, g.wp, g.fps
    for m in range(2):
        for (c0, nf) in MM_CHUNKS:
            po = fps.tile([128, 512], F32, tag="fps", bufs=2, name="ps_out")
            for cg in range(2):
                nc.tensor.matmul(po[:, :nf],
                                 lhsT=g.w_out_t[cg][:, m * 128:(m + 1) * 128],
                                 rhs=g.yv[cg][:, c0:c0 + nf],
                                 start=(cg == 0), stop=(cg == 1))
            osb = wp.tile([128, 512], F32, tag="osb", bufs=2)
            nc.scalar.activation(out=osb[:, :nf], in_=po[:, :nf], func=AF.Copy)
            nc.sync.dma_start(out=g.out_d[m * 128:(m + 1) * 128, c0:c0 + nf],
                              in_=osb[:, :nf])


def _r32r(a):
    """Round fp32 -> fp32r (TF32-like, 10 explicit mantissa bits)."""
    b = np.ascontiguousarray(a, np.float32).view(np.uint32)
    return (((b.astype(np.uint64) + 0x1000) & 0xFFFFE000)
            .astype(np.uint32).view(np.float32))


def _bf16(a):
    return np.ascontiguousarray(np.asarray(a, np.float32)).astype(
        ml_dtypes.bfloat16)


def _host_prep(inputs):
    x = np.asarray(inputs["x"], np.float32)
    W_pos = np.asarray(inputs["W_pos"], np.float32)
    b_pos = np.asarray(inputs["b_pos"], np.float32)
    W_in = np.asarray(inputs["W_in"], np.float32)
    pw1_w = np.asarray(inputs["pw1_w"], np.float32)
    pw1_b = np.asarray(inputs["pw1_b"], np.float32)
    dw_w = np.asarray(inputs["dw_w"], np.float32)
    pw2_w = np.asarray(inputs["pw2_w"], np.float32)
    W_xproj = np.asarray(inputs["W_xproj"], np.float32)
    W_dt = np.asarray(inputs["W_dt"], np.float32)
    b_dt = np.asarray(inputs["b_dt"], np.float32)
    A_log = np.asarray(inputs["A_log"], np.float32)
    Dp = np.asarray(inputs["Dp"], np.float32)
    dir_emb = np.asarray(inputs["dir_emb"], np.float32)
    W_out = np.asarray(inputs["W_out"], np.float32)

    gy, gx = np.meshgrid(np.arange(H, dtype=np.float32),
                         np.arange(W, dtype=np.float32), indexing="ij")
    coords = np.stack([gy, gx], -1) / (H - 1) * 2 - 1
    pos = (coords.reshape(L, 2) @ W_pos + b_pos).astype(np.float32)

    common = {
        "w_pw1": _bf16(pw1_w.reshape(MID, E).T),
        "pw1b": np.ascontiguousarray(pw1_b.reshape(MID, 1)),
        "dwtap": np.ascontiguousarray(dw_w.reshape(MID, 9)),
    }
    w_pw2_base = pw2_w.reshape(E, MID).T  # (MID, E)
    A = -np.exp(A_log)  # (E, N)
    xp = (x + pos[None]).transpose(0, 2, 1)  # (B, Dm, L)

    sel = np.zeros((2 * ESH, ESH), np.float32)
    for p in range(2 * ESH):
        sel[p, p % ESH] = 1.0
    sel = sel.astype(ml_dtypes.bfloat16)

    in_maps = []
    for c in range(NCORES):
        bcr = c // 4               # this core's batch
        e0 = (c % 4) * 2 * ESH     # this core's 128-channel slice
        sl = slice(e0, e0 + 2 * ESH)
        ascale = np.empty((2, 2 * ESH, NJ), np.float32)
        for cg in range(2):
            A_cg = A[e0 + cg * ESH:e0 + (cg + 1) * ESH]  # (64, 16)
            for p in range(2 * ESH):
                for j in range(NJ):
                    ascale[cg, p, j] = A_cg[p % ESH, 2 * j + p // ESH]
        m = dict(common)
        m["xT"] = _bf16(xp[bcr])
        # channel permutation putting this core's slice at rows [0:128]
        perm = np.concatenate([np.arange(e0, e0 + 2 * ESH),
                               np.arange(0, e0),
                               np.arange(e0 + 2 * ESH, E)])
        m["w_pw2"] = _bf16(w_pw2_base[:, perm])
        m["w_xp"] = _bf16(np.concatenate(
            [W_xproj[perm, :], np.zeros((E, 64 - (R + 2 * N)), np.float32)],
            axis=1))
        m["w_in"] = _bf16(
            np.concatenate([W_in[:, :E], W_in[:, E + e0:E + e0 + 2 * ESH]],
                           axis=1))
        m["w_dt"] = _bf16(W_dt[:, sl])
        m["spb"] = np.ascontiguousarray(
            (2.0 * b_dt[sl]).reshape(2, ESH, 1))
        m["ascale"] = ascale
        m["dire"] = np.ascontiguousarray(
            dir_emb[:, sl].T.reshape(2, ESH, 4))
        m["dp4"] = np.ascontiguousarray((4.0 * Dp[sl]).reshape(2, ESH, 1))
        m["dpb"] = np.ascontiguousarray(
            (Dp[sl] * dir_emb[:, sl].sum(0)).reshape(2, ESH, 1))
        m["w_out"] = _bf16(W_out[sl, :].reshape(2, ESH, DM))
        m["sel"] = sel
        in_maps.append(m)
    return in_maps


_PROGRAM = None
_LAST_RESULTS = None
_LAST_INSTS = None


def _get_program():
    global _PROGRAM
    if _PROGRAM is None:
        _PROGRAM = build_program()
    return _PROGRAM


def kernel(**inputs):
    global _LAST_EXEC_NS, _LAST_RESULTS
    assert int(inputs["H"]) == H and int(inputs["W"]) == W
    in_maps = _host_prep(inputs)
    if TRACE:
        _install_profile_shim()
    res = run_bass_kernel_spmd(_get_program(), in_maps,
                               list(range(NCORES)), trace=TRACE)
    _LAST_EXEC_NS = res.exec_time_ns
    _LAST_RESULTS = res.results
    global _LAST_INSTS
    _LAST_INSTS = res.instructions_and_trace
    out = np.zeros((B, DM, L), np.float32)
    for c, r in enumerate(res.results):
        out[c // 4] += np.asarray(r["out"], np.float32)
    return np.ascontiguousarray(out.transpose(0, 2, 1))


# revision 39
# speedup vs baseline: 1.0175x; 1.0045x over previous
"""Trainium2 Bass kernel for the LIDAR2D 4-direction selective-scan block.

Sharding: (batch, d_inner/4). Core c handles batch c//4 and a 128-channel
slice of d_inner (E=512): the host passes x[b] and channel-sliced scan
params per core, so the program stays SPMD-identical. Each core computes
the full-E front for its batch once, then scans its 128 channels as two
64-channel groups (cg) x 16 states x 4 directions, and emits a partial
out-projection (Dm, L). The host sums 4 partials per batch.

Scan layout per core: tiles of [128 partitions = (nsub in {0,1}) x (64
channels), free = L] — 8 tiles j=0..7 cover states n = 2j + nsub. The
recurrence h_t = exp(A*delta_t)*h_{t-1} + delta_t*B_t*u_t runs in a single
DVE tensor_tensor_scan per tile (2 cyc/elem — the hardware floor and the
kernel's critical path). y_t = C_t . h_t is a PE matmul with a 0/1
selection matrix contracting the two nsub rows per channel, accumulated
over j in PSUM. Direction permutations (reverse / spatial transpose) are
pure access-pattern tricks on reads/writes.

Perf structure: the front runs in bf16 (1 PE pass/col) with per-512-chunk
transient tiles. With one batch per core there is no second front to
overlap; the DVE stream is the 64 scans plus their elementwise feeds, and
the PE/Act fronts run only in the ~100us head.
"""

import os
import sys

for _p in ("/opt/trn_rl_repo", os.path.expanduser("~/.axon_site/_ro/trn_rl_repo")):
    if os.path.isdir(_p) and _p not in sys.path:
        sys.path.insert(0, _p)

import numpy as np
import ml_dtypes

import concourse.bass as bass
import concourse.bacc as bacc
import concourse.mybir as mybir
from concourse.tile import TileContext
from concourse.bass_utils import run_bass_kernel_spmd

F32 = mybir.dt.float32
F32R = mybir.dt.float32r
BF16 = mybir.dt.bfloat16
AF = mybir.ActivationFunctionType
OP = mybir.AluOpType

# Problem shape (hardcoded per the harness contract).
B, L, DM, E, N, R, MID, H, W = 2, 2304, 256, 512, 16, 16, 32, 48, 48
NCORES = 8
ESH = E // NCORES          # 64 channels per core
NJ = N // 2                # 8 scan tiles per (b, k); rows = (nsub, e_loc)
HALF = L // 2              # 1152, for PSUM-sized y accumulation

TRACE = bool(os.environ.get("KERNEL_TRACE"))
_LAST_EXEC_NS = None


def _install_profile_shim():
    """Make run_bass_kernel_spmd(trace=True) work in this container:
    register the NTFF hook (antenv.axon_hooks is absent here) and stub
    the S3 artifact upload."""
    import types
    try:
        from antenv.axon_hooks import get_axon_ntff_profile_hook  # noqa: F401
    except ImportError:
        import antenv
        mod = types.ModuleType("antenv.axon_hooks")
        mod._HOOK = None
        mod.set_axon_ntff_profile_hook = lambda h: setattr(mod, "_HOOK", h)
        mod.get_axon_ntff_profile_hook = lambda: mod._HOOK
        sys.modules["antenv.axon_hooks"] = mod
        antenv.axon_hooks = mod
        try:
            from trn_agent_boot.trn_boot import _ntff_profile_via_ctypes
            hook = _ntff_profile_via_ctypes("/opt/axon/libaxon_pjrt.so")
            if hook is not None:
                mod._HOOK = hook
        except Exception as e:  # pragma: no cover
            print(f"profile shim: hook install failed: {e}")
    import concourse.bass_utils as bu
    bu.upload_artifacts = lambda tmpdir: f"file://{tmpdir}"


def _chunks(total, step):
    out = []
    c0 = 0
    while c0 < total:
        out.append((c0, min(step, total - c0)))
        c0 += step
    return out


MM_CHUNKS = _chunks(L, 512)          # matmul free-dim chunks over full L
MM_CHUNKS_HALF = _chunks(HALF, 512)  # chunks within a 1152 half


class Env:
    pass


def build_program():
    nc = bacc.Bacc()
    g = Env()
    g.nc = nc

    # ---- DRAM parameters (same shapes on every core; values differ) ----
    g.xT_d = nc.declare_dram_parameter("xT", [DM, L], BF16, isOutput=False)
    g.w_in_d = nc.declare_dram_parameter("w_in", [DM, E + 2 * ESH], BF16, isOutput=False)
    g.w_pw1_d = nc.declare_dram_parameter("w_pw1", [E, MID], BF16, isOutput=False)
    g.pw1b_d = nc.declare_dram_parameter("pw1b", [MID, 1], F32, isOutput=False)
    g.dwtap_d = nc.declare_dram_parameter("dwtap", [MID, 9], F32, isOutput=False)
    g.w_pw2_d = nc.declare_dram_parameter("w_pw2", [MID, E], BF16, isOutput=False)
    g.w_xp_d = nc.declare_dram_parameter("w_xp", [E, 64], BF16, isOutput=False)
    g.w_dt_d = nc.declare_dram_parameter("w_dt", [R, 2 * ESH], BF16, isOutput=False)
    g.spb_d = nc.declare_dram_parameter("spb", [2, ESH, 1], F32, isOutput=False)
    g.ascale_d = nc.declare_dram_parameter("ascale", [2, 2 * ESH, NJ], F32, isOutput=False)
    g.dire_d = nc.declare_dram_parameter("dire", [2, ESH, 4], F32, isOutput=False)
    g.dp4_d = nc.declare_dram_parameter("dp4", [2, ESH, 1], F32, isOutput=False)
    g.dpb_d = nc.declare_dram_parameter("dpb", [2, ESH, 1], F32, isOutput=False)
    g.w_out_d = nc.declare_dram_parameter("w_out", [2, ESH, DM], BF16, isOutput=False)
    g.sel_d = nc.declare_dram_parameter("sel", [2 * ESH, ESH], BF16, isOutput=False)
    g.out_d = nc.declare_dram_parameter("out", [DM, L], F32, isOutput=True)

    with TileContext(nc) as tc:
        g.tc = tc
        with tc.tile_pool(name="const", bufs=1) as cp, \
             tc.tile_pool(name="persist", bufs=1) as pp, \
             tc.tile_pool(name="front", bufs=1) as fp, \
             tc.tile_pool(name="scan", bufs=1) as sp, \
             tc.tile_pool(name="work", bufs=2) as wp, \
             tc.tile_pool(name="fpsum", bufs=2, space="PSUM") as fps, \
             tc.tile_pool(name="ypsum", bufs=1, space="PSUM") as yps, \
             tc.tile_pool(name="bounce", bufs=1, space="DRAM") as bp:
            g.cp, g.pp, g.fp, g.sp, g.wp, g.fps, g.yps, g.bp = \
                cp, pp, fp, sp, wp, fps, yps, bp
            _load_consts(g)

            # persistent per-b products of the front
            g.xc_sl = [pp.tile([ESH, L], BF16, tag=f"xc_sl{b}", name=f"xc_sl{b}") for b in range(B)]
            g.delta_rep = [pp.tile([128, L], F32, tag=f"drep{b}", name=f"drep{b}") for b in range(B)]
            g.z_sl = [pp.tile([ESH, L], BF16, tag=f"z{b}", name=f"z{b}") for b in range(B)]
            g.y_acc = [pp.tile([ESH, L], F32, tag=f"yacc{b}", name=f"yacc{b}") for b in range(B)]
            g.bsrc = bp.tile([N, L], BF16, tag="bsrc", name="bsrc")
            g.csrc = bp.tile([N, L], BF16, tag="csrc", name="csrc")
            # h12/acc full-L per-b (conv needs the whole plane)
            g.h12 = fp.tile([MID, L], BF16, tag="h12", name="h12")
            g.accB = fp.tile([MID, L], BF16, tag="accB", name="accB")
            g.xdbl = fp.tile([64, L], BF16, tag="xdbl", name="xdbl")
            g.dtlin = [fp.tile([ESH, L], BF16, tag=f"dtlin{c}", name=f"dtlin{c}") for c in range(2)]
            g.dA = [None] * NJ
            g.yv = [None, None]

            # ---- emission schedule (one batch per core, two cg groups) ----
            _front_stageA(g)
            _front_conv(g)
            _front_stageB(g)
            _scan_prep(g, 0)
            du = _mk_du(g, 0, 0)
            pend, du = _scan_k(g, 0, 0, du, nxt=(0, 1))
            _stage_z(g)
            for k in range(1, 4):
                nxt = (0, k + 1) if k < 3 else (1, 0)
                pend, du = _scan_k(g, 0, k, du, nxt=nxt, pending=pend)
            _scan_prep(g, 1)
            pend()
            _finish_yv(g, 0)
            pend = None
            for k in range(4):
                nxt = (1, k + 1) if k < 3 else None
                pend, du = _scan_k(g, 1, k, du, nxt=nxt, pending=pend)
            pend()
            _finish_yv(g, 1)
            _out_proj(g)

    nc.finalize()
    return nc


def _load_consts(g):
    nc, cp = g.nc, g.cp
    g.w_in_t = [cp.tile([128, E + 2 * ESH], BF16, tag=f"w_in{t}", name=f"w_in{t}") for t in range(2)]
    for t in range(2):
        nc.sync.dma_start(out=g.w_in_t[t][:], in_=g.w_in_d[t * 128:(t + 1) * 128, :])
    g.w_pw1_t = [cp.tile([128, MID], BF16, tag=f"w_pw1{t}", name=f"w_pw1{t}") for t in range(4)]
    for t in range(4):
        nc.sync.dma_start(out=g.w_pw1_t[t][:], in_=g.w_pw1_d[t * 128:(t + 1) * 128, :])
    g.pw1b_t = cp.tile([MID, 1], F32, tag="pw1b", name="pw1b")
    nc.sync.dma_start(out=g.pw1b_t[:], in_=g.pw1b_d[:])
    g.dwtap_t = cp.tile([MID, 9], F32, tag="dwtap", name="dwtap")
    nc.sync.dma_start(out=g.dwtap_t[:], in_=g.dwtap_d[:])
    g.w_pw2_t = cp.tile([MID, E], BF16, tag="w_pw2", name="w_pw2")
    nc.sync.dma_start(out=g.w_pw2_t[:], in_=g.w_pw2_d[:])
    g.w_xp_t = [cp.tile([128, 64], BF16, tag=f"w_xp{t}", name=f"w_xp{t}") for t in range(4)]
    for t in range(4):
        nc.sync.dma_start(out=g.w_xp_t[t][:], in_=g.w_xp_d[t * 128:(t + 1) * 128, :])
    g.w_dt_t = cp.tile([R, 2 * ESH], BF16, tag="w_dt", name="w_dt")
    nc.sync.dma_start(out=g.w_dt_t[:], in_=g.w_dt_d[:])
    g.spb_t = [cp.tile([ESH, 1], F32, tag=f"spb{c}", name=f"spb{c}") for c in range(2)]
    g.ascale_t = [cp.tile([2 * ESH, NJ], F32, tag=f"ascale{c}", name=f"ascale{c}") for c in range(2)]
    g.dire_t = [cp.tile([ESH, 4], F32, tag=f"dire{c}", name=f"dire{c}") for c in range(2)]
    g.dp4_t = [cp.tile([ESH, 1], F32, tag=f"dp4{c}", name=f"dp4{c}") for c in range(2)]
    g.dpb_t = [cp.tile([ESH, 1], F32, tag=f"dpb{c}", name=f"dpb{c}") for c in range(2)]
    g.w_out_t = [cp.tile([ESH, DM], BF16, tag=f"w_out{c}", name=f"w_out{c}") for c in range(2)]
    for c in range(2):
        nc.sync.dma_start(out=g.spb_t[c][:], in_=g.spb_d[c])
        nc.sync.dma_start(out=g.ascale_t[c][:], in_=g.ascale_d[c])
        nc.sync.dma_start(out=g.dire_t[c][:], in_=g.dire_d[c])
        nc.sync.dma_start(out=g.dp4_t[c][:], in_=g.dp4_d[c])
        nc.sync.dma_start(out=g.dpb_t[c][:], in_=g.dpb_d[c])
        nc.sync.dma_start(out=g.w_out_t[c][:], in_=g.w_out_d[c])
    g.sel_t = cp.tile([2 * ESH, ESH], BF16, tag="sel", name="sel")
    nc.sync.dma_start(out=g.sel_t[:], in_=g.sel_d[:])


def _front_stageA(g):
    """x+pos -> xz -> (z slices, xh) -> pw1 -> h12 (full [MID, L])."""
    nc, fp, fps = g.nc, g.fp, g.fps
    for (c0, nf) in MM_CHUNKS:
        xin = [fp.tile([128, 512], BF16, tag=f"xin{t}", bufs=2, name=f"xin{t}") for t in range(2)]
        for t in range(2):
            nc.sync.dma_start(out=xin[t][:, :nf],
                              in_=g.xT_d[t * 128:(t + 1) * 128, c0:c0 + nf])
        xhc = [fp.tile([128, 512], BF16, tag=f"xhc{m}", bufs=2, name=f"xhc{m}") for m in range(4)]
        for m in range(4):
            ps = fps.tile([128, 512], F32, tag="fps", bufs=2, name="ps_xz")
            for kt in range(2):
                nc.tensor.matmul(ps[:, :nf],
                                 lhsT=g.w_in_t[kt][:, m * 128:(m + 1) * 128],
                                 rhs=xin[kt][:, :nf],
                                 start=(kt == 0), stop=(kt == 1))
            nc.scalar.activation(out=xhc[m][:, :nf], in_=ps[:, :nf], func=AF.Copy)
        ps1 = fps.tile([MID, 512], F32, tag="fps", bufs=2, name="ps_pw1")
        for kt in range(4):
            nc.tensor.matmul(ps1[:, :nf], lhsT=g.w_pw1_t[kt][:],
                             rhs=xhc[kt][:, :nf],
                             start=(kt == 0), stop=(kt == 3))
        nc.scalar.activation(out=g.h12[:, c0:c0 + nf], in_=ps1[:, :nf],
                             func=AF.Identity, bias=g.pw1b_t[:])


def _stage_z(g):
    """z projection, deferred out of the head: re-DMAs x chunks (reusing
    stageA's xin buffers) and runs on the PE/Act slack under scan(cg0)."""
    nc, fp, fps = g.nc, g.fp, g.fps
    for (c0, nf) in MM_CHUNKS:
        xin = [fp.tile([128, 512], BF16, tag=f"xin{t}", bufs=2, name=f"zin{t}") for t in range(2)]
        for t in range(2):
            nc.sync.dma_start(out=xin[t][:, :nf],
                              in_=g.xT_d[t * 128:(t + 1) * 128, c0:c0 + nf])
        for cg in range(2):
            psz = fps.tile([ESH, 512], F32, tag="fps", bufs=2, name="ps_z")
            for kt in range(2):
                nc.tensor.matmul(psz[:, :nf],
                                 lhsT=g.w_in_t[kt][:, E + cg * ESH:E + (cg + 1) * ESH],
                                 rhs=xin[kt][:, :nf],
                                 start=(kt == 0), stop=(kt == 1))
            nc.scalar.activation(out=g.z_sl[cg][:, c0:c0 + nf], in_=psz[:, :nf],
                                 func=AF.Copy)


def _front_conv(g):
    """Depthwise 3x3 on h12 -> accB, in 4 row-bands so each band's ops
    start as soon as the h12 chunks covering its rows (+1 halo) land."""
    nc, fp = g.nc, g.fp
    acc = fp.tile([MID, L], BF16, tag="dwacc")
    acc3 = acc[:].rearrange("p (h w) -> p h w", w=W)
    h3 = g.h12[:].rearrange("p (h w) -> p h w", w=W)
    BAND = 12
    for b0 in range(0, H, BAND):
        b1 = b0 + BAND
        nc.vector.tensor_scalar(out=acc3[:, b0:b1, :],
                                in0=h3[:, b0:b1, :],
                                scalar1=g.dwtap_t[:, 4:5], scalar2=None,
                                op0=OP.mult)
        for ky in range(3):
            for kx in range(3):
                if ky == 1 and kx == 1:
                    continue
                dy, dx = ky - 1, kx - 1
                r0 = max(max(0, -dy), b0)
                r1 = min(H - max(0, dy), b1)
                if r0 >= r1:
                    continue
                w0, w1 = max(0, -dx), W - max(0, dx)
                nc.vector.scalar_tensor_tensor(
                    out=acc3[:, r0:r1, w0:w1],
                    in0=h3[:, r0 + dy:r1 + dy, w0 + dx:w1 + dx],
                    scalar=g.dwtap_t[:, ky * 3 + kx:ky * 3 + kx + 1],
                    in1=acc3[:, r0:r1, w0:w1],
                    op0=OP.mult, op1=OP.add)
        nc.scalar.activation(out=g.accB[:, b0 * W:b1 * W],
                             in_=acc[:, b0 * W:b1 * W], func=AF.Copy)


def _front_stageB(g):
    """pw2+SiLU -> xc chunks -> (xc_sl slice, x_dbl, delta, B/C to DRAM)."""
    nc, fp, fps = g.nc, g.fp, g.fps
    for (c0, nf) in MM_CHUNKS:
        xcc = [fp.tile([128, 512], BF16, tag=f"xcc{m}", bufs=2, name=f"xcc{m}") for m in range(4)]
        for m in range(4):
            ps2 = fps.tile([128, 512], F32, tag="fps", bufs=2, name="ps_pw2")
            nc.tensor.matmul(ps2[:, :nf],
                             lhsT=g.w_pw2_t[:, m * 128:(m + 1) * 128],
                             rhs=g.accB[:, c0:c0 + nf], start=True, stop=True)
            nc.scalar.activation(out=xcc[m][:, :nf], in_=ps2[:, :nf],
                                 func=AF.Silu)
        for cg in range(2):
            nc.sync.dma_start(out=g.xc_sl[cg][:, c0:c0 + nf],
                              in_=xcc[0][cg * ESH:(cg + 1) * ESH, :nf])
        ps3 = fps.tile([64, 512], F32, tag="fps", bufs=2, name="ps_xdbl")
        for kt in range(4):
            nc.tensor.matmul(ps3[:, :nf], lhsT=g.w_xp_t[kt][:],
                             rhs=xcc[kt][:, :nf],
                             start=(kt == 0), stop=(kt == 3))
        nc.scalar.activation(out=g.xdbl[:, c0:c0 + nf], in_=ps3[:, :nf],
                             func=AF.Copy)
        for cg in range(2):
            ps4 = fps.tile([ESH, 512], F32, tag="fps", bufs=2, name="ps_dt")
            nc.tensor.matmul(ps4[:, :nf],
                             lhsT=g.w_dt_t[:, cg * ESH:(cg + 1) * ESH],
                             rhs=g.xdbl[0:R, c0:c0 + nf], start=True, stop=True)
            nc.scalar.activation(out=g.dtlin[cg][:, c0:c0 + nf], in_=ps4[:, :nf],
                                 func=AF.Copy)
    # softplus(v) = ln(1 + exp(v)); |v| < ~10 here so exp cannot overflow.
    # Batched over full L to avoid ACT table reloads inside the chunk loop.
    for cg in range(2):
        nc.scalar.activation(out=g.delta_rep[cg][0:ESH, :], in_=g.dtlin[cg][:],
                             func=AF.Exp, bias=g.spb_t[cg][:])
        nc.scalar.activation(out=g.delta_rep[cg][0:ESH, :],
                             in_=g.delta_rep[cg][0:ESH, :], func=AF.Ln, bias=1.0)
        # duplicate delta rows [0:64] -> [64:128]
        nc.sync.dma_start(out=g.delta_rep[cg][ESH:2 * ESH, :],
                          in_=g.delta_rep[cg][0:ESH, :])
    # B/C rows to DRAM for later partition-broadcast loads
    nc.sync.dma_start(out=g.bsrc[:], in_=g.xdbl[R:R + N, :])
    nc.sync.dma_start(out=g.csrc[:], in_=g.xdbl[R + N:R + 2 * N, :])


def _scan_prep(g, cg):
    """y_acc init (D*u skip), fused-pair dA exps (with zero seam)."""
    nc, sp, wp = g.nc, g.sp, g.wp
    nc.scalar.activation(out=g.y_acc[cg][:], in_=g.xc_sl[cg][:],
                         func=AF.Identity, bias=g.dpb_t[cg][:],
                         scale=g.dp4_t[cg][:])
    for jp in range(NJ // 2):
        # dA for pair (2jp, 2jp+1) fused along free dim; the seam column
        # (fused position L = j-odd's t=0) is zeroed so the recurrence
        # restarts: h = 0*h_prev + dbu = the correct fresh-scan init.
        g.dA[jp] = sp.tile([128, 2 * L], BF16, tag=f"dA{jp}", name=f"dA{jp}")
        for s in range(2):
            nc.scalar.activation(out=g.dA[jp][:, s * L:(s + 1) * L],
                                 in_=g.delta_rep[cg][:], func=AF.Exp,
                                 scale=g.ascale_t[cg][:, 2 * jp + s:2 * jp + s + 1])
        nc.vector.memset(g.dA[jp][:, L:L + 1], 0.0)



def _mk_du(g, cg, k):
    """u_k = perm_k(xc)+dir_k (Act); du = delta*u (DVE); ns-dup DMA.
    Emitted inside the previous direction's last pair so the dup DMA
    latency hides under that pair's hc work."""
    nc, wp = g.nc, g.wp
    xc3 = g.xc_sl[cg][:].rearrange("p (h w) -> p h w", w=W)
    xcT = g.xc_sl[cg][:].rearrange("p (h w) -> p w h", w=W)
    usrc = [xc3, xc3[:, ::-1, ::-1], xcT, xcT[:, ::-1, ::-1]][k]
    u_tmp = wp.tile([ESH, L], BF16, tag="u_tmp", bufs=1)
    u3 = u_tmp[:].rearrange("p (a c) -> p a c", c=W)
    nc.scalar.activation(out=u3, in_=usrc, func=AF.Identity,
                         bias=g.dire_t[cg][:, k:k + 1])
    du = wp.tile([128, L], BF16, tag="du", bufs=2)
    nc.vector.tensor_tensor(out=du[0:ESH, :], in0=g.delta_rep[cg][0:ESH, :],
                            in1=u_tmp[:], op=OP.mult)
    nc.sync.dma_start(out=du[ESH:2 * ESH, :], in_=du[0:ESH, :])
    return du


def _scan_k(g, cg, k, du, nxt=None, pending=None):
    """One direction's 8 scan tiles (scans fused in j-pairs). `nxt`
    (cg, k) identifies the following direction whose du block is emitted
    inside this direction's last pair. Returns (drain closure, next du)."""
    nc, wp, yps = g.nc, g.wp, g.yps
    du_next = None
    ypsum = [yps.tile([ESH, HALF], F32, tag=f"yps{h}", name=f"yps{h}") for h in range(2)]
    for jp in range(NJ // 2):
        dbu = wp.tile([128, 2 * L], BF16, tag="workA", bufs=1)
        for s in range(2):
            B_t = wp.tile([128, L], BF16, tag="B_t", bufs=2)
            C_s = [None, None]
            for ns in range(2):
                row = 4 * jp + 2 * s + ns
                nc.sync.dma_start(
                    out=B_t[ns * ESH:(ns + 1) * ESH, :],
                    in_=g.bsrc[row:row + 1, :].to_broadcast((ESH, L)))
            nc.vector.tensor_tensor(out=dbu[:, s * L:(s + 1) * L],
                                    in0=du[:], in1=B_t[:], op=OP.mult)
        h_t = wp.tile([128, 2 * L], BF16, tag="workH", bufs=1)
        nc.vector.tensor_tensor_scan(out=h_t[:], data0=g.dA[jp][:],
                                     data1=dbu[:], initial=0.0,
                                     op0=OP.mult, op1=OP.add)
        if jp == 0 and pending is not None:
            # previous direction's PSUM drain: emitted here (before this
            # direction's first start=True sel-matmul touches ypsum) but
            # after ~12us of fused dbu+scan, so it no longer stalls the
            # DVE on the PE's last sel-matmul of the previous direction.
            pending()
        if jp == NJ // 2 - 1 and nxt is not None:
            du_next = _mk_du(g, *nxt)
        for s in range(2):
            C_t = wp.tile([128, L], BF16, tag="C_t", bufs=2)
            for ns in range(2):
                row = 4 * jp + 2 * s + ns
                nc.sync.dma_start(
                    out=C_t[ns * ESH:(ns + 1) * ESH, :],
                    in_=g.csrc[row:row + 1, :].to_broadcast((ESH, L)))
            hc = wp.tile([128, L], BF16, tag="workB", bufs=2)
            nc.vector.tensor_tensor(out=hc[:], in0=h_t[:, s * L:(s + 1) * L],
                                    in1=C_t[:], op=OP.mult)
            for hh in range(2):
                for (c0, nf) in MM_CHUNKS_HALF:
                    nc.tensor.matmul(
                        ypsum[hh][:, c0:c0 + nf],
                        lhsT=g.sel_t[:],
                        rhs=hc[:, hh * HALF + c0:hh * HALF + c0 + nf],
                        start=(jp == 0 and s == 0),
                        stop=(jp == NJ // 2 - 1 and s == 1))
    def drain():
        # accumulate un-permuted ys_k into y_acc
        for hh in range(2):
            pv = ypsum[hh][:]
            if k == 0:
                dst = g.y_acc[cg][:, hh * HALF:(hh + 1) * HALF]
                srcv = pv
            elif k == 1:
                dst = g.y_acc[cg][:, (1 - hh) * HALF:(2 - hh) * HALF]
                srcv = pv[:, ::-1]
            elif k == 2:
                # ys[i], i=a*48+b_ -> l = b_*48+a ; half hh: a in [24hh,...)
                dst = g.y_acc[cg][:].rearrange("p (bb a) -> p bb a", a=W)[
                    :, :, 24 * hh:24 * hh + 24]
                srcv = pv.rearrange("p (a bb) -> p bb a", bb=W)
            else:
                dst = g.y_acc[cg][:].rearrange("p (bb a) -> p bb a", a=W)[
                    :, :, 24 * (1 - hh):24 * (1 - hh) + 24]
                srcv = pv.rearrange("p (a bb) -> p bb a", bb=W)[:, ::-1, ::-1]
            nc.vector.tensor_tensor(out=dst, in0=srcv, in1=dst, op=OP.add)
    return drain, du_next


def _finish_yv(g, cg):
    """yv[cg] = y_acc * silu(z) for this channel group."""
    nc, wp = g.nc, g.wp
    sz = wp.tile([ESH, L], BF16, tag="u_tmp", bufs=1)
    nc.scalar.activation(out=sz[:], in_=g.z_sl[cg][:], func=AF.Silu)
    g.yv[cg] = g.fp.tile([ESH, L], BF16, tag=f"dtlin{cg}", name=f"yv{cg}")
    nc.vector.tensor_tensor(out=g.yv[cg][:], in0=g.y_acc[cg][:], in1=sz[:],
                            op=OP.mult)


def _out_proj(g):
    """out_partial = sum_cg W_out[cg]^T @ yv[cg] (PSUM-accumulated)."""
    nc, wp, fps = g.nc, g.wp, g.fps
    for m in range(2):
        for (c0, nf) in MM_CHUNKS:
            po = fps.tile([128, 512], F32, tag="fps", bufs=2, name="ps_out")
            for cg in range(2):
                nc.tensor.matmul(po[:, :nf],
                                 lhsT=g.w_out_t[cg][:, m * 128:(m + 1) * 128],
                                 rhs=g.yv[cg][:, c0:c0 + nf],
                                 start=(cg == 0), stop=(cg == 1))
            osb = wp.tile([128, 512], F32, tag="osb", bufs=2)
            nc.scalar.activation(out=osb[:, :nf], in_=po[:, :nf], func=AF.Copy)
            nc.sync.dma_start(out=g.out_d[m * 128:(m + 1) * 128, c0:c0 + nf],
                              in_=osb[:, :nf])


def _r32r(a):
    """Round fp32 -> fp32r (TF32-like, 10 explicit mantissa bits)."""
    b = np.ascontiguousarray(a, np.float32).view(np.uint32)
    return (((b.astype(np.uint64) + 0x1000) & 0xFFFFE000)
            .astype(np.uint32).view(np.float32))


def _bf16(a):
    return np.ascontiguousarray(np.asarray(a, np.float32)).astype(
        ml_dtypes.bfloat16)


def _host_prep(inputs):
    x = np.asarray(inputs["x"], np.float32)
    W_pos = np.asarray(inputs["W_pos"], np.float32)
    b_pos = np.asarray(inputs["b_pos"], np.float32)
    W_in = np.asarray(inputs["W_in"], np.float32)
    pw1_w = np.asarray(inputs["pw1_w"], np.float32)
    pw1_b = np.asarray(inputs["pw1_b"], np.float32)
    dw_w = np.asarray(inputs["dw_w"], np.float32)
    pw2_w = np.asarray(inputs["pw2_w"], np.float32)
    W_xproj = np.asarray(inputs["W_xproj"], np.float32)
    W_dt = np.asarray(inputs["W_dt"], np.float32)
    b_dt = np.asarray(inputs["b_dt"], np.float32)
    A_log = np.asarray(inputs["A_log"], np.float32)
    Dp = np.asarray(inputs["Dp"], np.float32)
    dir_emb = np.asarray(inputs["dir_emb"], np.float32)
    W_out = np.asarray(inputs["W_out"], np.float32)

    gy, gx = np.meshgrid(np.arange(H, dtype=np.float32),
                         np.arange(W, dtype=np.float32), indexing="ij")
    coords = np.stack([gy, gx], -1) / (H - 1) * 2 - 1
    pos = (coords.reshape(L, 2) @ W_pos + b_pos).astype(np.float32)

    common = {
        "w_pw1": _bf16(pw1_w.reshape(MID, E).T),
        "pw1b": np.ascontiguousarray(pw1_b.reshape(MID, 1)),
        "dwtap": np.ascontiguousarray(dw_w.reshape(MID, 9)),
    }
    w_pw2_base = pw2_w.reshape(E, MID).T  # (MID, E)
    A = -np.exp(A_log)  # (E, N)
    xp = (x + pos[None]).transpose(0, 2, 1)  # (B, Dm, L)

    sel = np.zeros((2 * ESH, ESH), np.float32)
    for p in range(2 * ESH):
        sel[p, p % ESH] = 1.0
    sel = sel.astype(ml_dtypes.bfloat16)

    in_maps = []
    for c in range(NCORES):
        bcr = c // 4               # this core's batch
        e0 = (c % 4) * 2 * ESH     # this core's 128-channel slice
        sl = slice(e0, e0 + 2 * ESH)
        ascale = np.empty((2, 2 * ESH, NJ), np.float32)
        for cg in range(2):
            A_cg = A[e0 + cg * ESH:e0 + (cg + 1) * ESH]  # (64, 16)
            for p in range(2 * ESH):
                for j in range(NJ):
                    ascale[cg, p, j] = A_cg[p % ESH, 2 * j + p // ESH]
        m = dict(common)
        m["xT"] = _bf16(xp[bcr])
        # channel permutation putting this core's slice at rows [0:128]
        perm = np.concatenate([np.arange(e0, e0 + 2 * ESH),
                               np.arange(0, e0),
                               np.arange(e0 + 2 * ESH, E)])
        m["w_pw2"] = _bf16(w_pw2_base[:, perm])
        m["w_xp"] = _bf16(np.concatenate(
            [W_xproj[perm, :], np.zeros((E, 64 - (R + 2 * N)), np.float32)],
            axis=1))
        m["w_in"] = _bf16(
            np.concatenate([W_in[:, :E], W_in[:, E + e0:E + e0 + 2 * ESH]],
                           axis=1))
        m["w_dt"] = _bf16(W_dt[:, sl])
        m["spb"] = np.ascontiguousarray(
            (2.0 * b_dt[sl]).reshape(2, ESH, 1))
        m["ascale"] = ascale
        m["dire"] = np.ascontiguousarray(
            dir_emb[:, sl].T.reshape(2, ESH, 4))
        m["dp4"] = np.ascontiguousarray((4.0 * Dp[sl]).reshape(2, ESH, 1))
        m["dpb"] = np.ascontiguousarray(
            (Dp[sl] * dir_emb[:, sl].sum(0)).reshape(2, ESH, 1))
        m["w_out"] = _bf16(W_out[sl, :].reshape(2, ESH, DM))
        m["sel"] = sel
        in_maps.append(m)
    return in_maps


_PROGRAM = None
_LAST_RESULTS = None
_LAST_INSTS = None


def _get_program():
    global _PROGRAM
    if _PROGRAM is None:
        _PROGRAM = build_program()
    return _PROGRAM


def kernel(**inputs):
    global _LAST_EXEC_NS, _LAST_RESULTS
    assert int(inputs["H"]) == H and int(inputs["W"]) == W
    in_maps = _host_prep(inputs)
    if TRACE:
        _install_profile_shim()
    res = run_bass_kernel_spmd(_get_program(), in_maps,
                               list(range(NCORES)), trace=TRACE)
    _LAST_EXEC_NS = res.exec_time_ns
    _LAST_RESULTS = res.results
    global _LAST_INSTS
    _LAST_INSTS = res.instructions_and_trace
    out = np.zeros((B, DM, L), np.float32)
    for c, r in enumerate(res.results):
        out[c // 4] += np.asarray(r["out"], np.float32)
    return np.ascontiguousarray(out.transpose(0, 2, 1))


# revision 40
# speedup vs baseline: 1.0273x; 1.0096x over previous
"""Trainium2 Bass kernel for the LIDAR2D 4-direction selective-scan block.

Sharding: (batch, d_inner/4). Core c handles batch c//4 and a 128-channel
slice of d_inner (E=512): the host passes x[b] and channel-sliced scan
params per core, so the program stays SPMD-identical. Each core computes
the full-E front for its batch once, then scans its 128 channels as two
64-channel groups (cg) x 16 states x 4 directions, and emits a partial
out-projection (Dm, L). The host sums 4 partials per batch.

Scan layout per core: tiles of [128 partitions = (nsub in {0,1}) x (64
channels), free = L] — 8 tiles j=0..7 cover states n = 2j + nsub. The
recurrence h_t = exp(A*delta_t)*h_{t-1} + delta_t*B_t*u_t runs in a single
DVE tensor_tensor_scan per tile (2 cyc/elem — the hardware floor and the
kernel's critical path). y_t = C_t . h_t is a PE matmul with a 0/1
selection matrix contracting the two nsub rows per channel, accumulated
over j in PSUM. Direction permutations (reverse / spatial transpose) are
pure access-pattern tricks on reads/writes.

Perf structure: the front runs in bf16 (1 PE pass/col) with per-512-chunk
transient tiles. With one batch per core there is no second front to
overlap; the DVE stream is the 64 scans plus their elementwise feeds, and
the PE/Act fronts run only in the ~100us head.
"""

import os
import sys

for _p in ("/opt/trn_rl_repo", os.path.expanduser("~/.axon_site/_ro/trn_rl_repo")):
    if os.path.isdir(_p) and _p not in sys.path:
        sys.path.insert(0, _p)

import numpy as np
import ml_dtypes

import concourse.bass as bass
import concourse.bacc as bacc
import concourse.mybir as mybir
from concourse.tile import TileContext
from concourse.bass_utils import run_bass_kernel_spmd

F32 = mybir.dt.float32
F32R = mybir.dt.float32r
BF16 = mybir.dt.bfloat16
AF = mybir.ActivationFunctionType
OP = mybir.AluOpType

# Problem shape (hardcoded per the harness contract).
B, L, DM, E, N, R, MID, H, W = 2, 2304, 256, 512, 16, 16, 32, 48, 48
NCORES = 8
ESH = E // NCORES          # 64 channels per core
NJ = N // 2                # 8 scan tiles per (b, k); rows = (nsub, e_loc)
HALF = L // 2              # 1152, for PSUM-sized y accumulation

TRACE = bool(os.environ.get("KERNEL_TRACE"))
_LAST_EXEC_NS = None


def _install_profile_shim():
    """Make run_bass_kernel_spmd(trace=True) work in this container:
    register the NTFF hook (antenv.axon_hooks is absent here) and stub
    the S3 artifact upload."""
    import types
    try:
        from antenv.axon_hooks import get_axon_ntff_profile_hook  # noqa: F401
    except ImportError:
        import antenv
        mod = types.ModuleType("antenv.axon_hooks")
        mod._HOOK = None
        mod.set_axon_ntff_profile_hook = lambda h: setattr(mod, "_HOOK", h)
        mod.get_axon_ntff_profile_hook = lambda: mod._HOOK
        sys.modules["antenv.axon_hooks"] = mod
        antenv.axon_hooks = mod
        try:
            from trn_agent_boot.trn_boot import _ntff_profile_via_ctypes
            hook = _ntff_profile_via_ctypes("/opt/axon/libaxon_pjrt.so")
            if hook is not None:
                mod._HOOK = hook
        except Exception as e:  # pragma: no cover
            print(f"profile shim: hook install failed: {e}")
    import concourse.bass_utils as bu
    bu.upload_artifacts = lambda tmpdir: f"file://{tmpdir}"


def _chunks(total, step):
    out = []
    c0 = 0
    while c0 < total:
        out.append((c0, min(step, total - c0)))
        c0 += step
    return out


MM_CHUNKS = _chunks(L, 512)          # matmul free-dim chunks over full L
MM_CHUNKS_HALF = _chunks(HALF, 512)  # chunks within a 1152 half


class Env:
    pass


def build_program():
    nc = bacc.Bacc()
    g = Env()
    g.nc = nc

    # ---- DRAM parameters (same shapes on every core; values differ) ----
    g.xT_d = nc.declare_dram_parameter("xT", [DM, L], BF16, isOutput=False)
    g.w_in_d = nc.declare_dram_parameter("w_in", [DM, E + 2 * ESH], BF16, isOutput=False)
    g.w_pw1_d = nc.declare_dram_parameter("w_pw1", [E, MID], BF16, isOutput=False)
    g.pw1b_d = nc.declare_dram_parameter("pw1b", [MID, 1], F32, isOutput=False)
    g.dwtap_d = nc.declare_dram_parameter("dwtap", [MID, 9], F32, isOutput=False)
    g.w_pw2_d = nc.declare_dram_parameter("w_pw2", [MID, E], BF16, isOutput=False)
    g.w_xp_d = nc.declare_dram_parameter("w_xp", [E, 64], BF16, isOutput=False)
    g.w_dt_d = nc.declare_dram_parameter("w_dt", [R, 2 * ESH], BF16, isOutput=False)
    g.spb_d = nc.declare_dram_parameter("spb", [2, ESH, 1], F32, isOutput=False)
    g.ascale_d = nc.declare_dram_parameter("ascale", [2, 2 * ESH, NJ], F32, isOutput=False)
    g.dire_d = nc.declare_dram_parameter("dire", [2, ESH, 4], F32, isOutput=False)
    g.dp4_d = nc.declare_dram_parameter("dp4", [2, ESH, 1], F32, isOutput=False)
    g.dpb_d = nc.declare_dram_parameter("dpb", [2, ESH, 1], F32, isOutput=False)
    g.w_out_d = nc.declare_dram_parameter("w_out", [2, ESH, DM], BF16, isOutput=False)
    g.sel_d = nc.declare_dram_parameter("sel", [2 * ESH, ESH], BF16, isOutput=False)
    g.out_d = nc.declare_dram_parameter("out", [DM, L], F32, isOutput=True)

    with TileContext(nc) as tc:
        g.tc = tc
        with tc.tile_pool(name="const", bufs=1) as cp, \
             tc.tile_pool(name="persist", bufs=1) as pp, \
             tc.tile_pool(name="front", bufs=1) as fp, \
             tc.tile_pool(name="scan", bufs=1) as sp, \
             tc.tile_pool(name="work", bufs=2) as wp, \
             tc.tile_pool(name="fpsum", bufs=2, space="PSUM") as fps, \
             tc.tile_pool(name="ypsum", bufs=1, space="PSUM") as yps, \
             tc.tile_pool(name="bounce", bufs=1, space="DRAM") as bp:
            g.cp, g.pp, g.fp, g.sp, g.wp, g.fps, g.yps, g.bp = \
                cp, pp, fp, sp, wp, fps, yps, bp
            _load_consts(g)

            # persistent per-b products of the front
            g.xc_sl = [pp.tile([ESH, L], BF16, tag=f"xc_sl{b}", name=f"xc_sl{b}") for b in range(B)]
            g.delta_rep = [pp.tile([128, L], F32, tag=f"drep{b}", name=f"drep{b}") for b in range(B)]
            g.z_sl = [pp.tile([ESH, L], BF16, tag=f"z{b}", name=f"z{b}") for b in range(B)]
            g.y_acc = [pp.tile([ESH, L], F32, tag=f"yacc{b}", name=f"yacc{b}") for b in range(B)]
            g.bsrc = bp.tile([N, L], BF16, tag="bsrc", name="bsrc")
            g.csrc = bp.tile([N, L], BF16, tag="csrc", name="csrc")
            # h12/acc full-L per-b (conv needs the whole plane)
            g.h12 = fp.tile([MID, L], BF16, tag="h12", name="h12")
            g.accB = fp.tile([MID, L], BF16, tag="accB", name="accB")
            g.xdbl = fp.tile([64, L], BF16, tag="xdbl", name="xdbl")
            g.dtlin = [fp.tile([ESH, L], BF16, tag=f"dtlin{c}", name=f"dtlin{c}") for c in range(2)]
            g.dA = [None] * NJ
            g.yv = [None, None]

            # ---- emission schedule (one batch per core, two cg groups) ----
            _front_stageA(g)
            _front_conv(g)
            _front_stageB(g)
            _scan_prep(g, 0)
            du = _mk_du(g, 0, 0)
            pend, du = _scan_k(g, 0, 0, du, nxt=(0, 1))
            _stage_z(g)
            for k in range(1, 4):
                nxt = (0, k + 1) if k < 3 else (1, 0)
                pend, du = _scan_k(g, 0, k, du, nxt=nxt, pending=pend)
            _scan_prep(g, 1)
            pend()
            _finish_yv(g, 0)
            pend = None
            for k in range(4):
                nxt = (1, k + 1) if k < 3 else None
                pend, du = _scan_k(g, 1, k, du, nxt=nxt, pending=pend)
            pend()
            _finish_yv(g, 1)
            _out_proj(g)

    nc.finalize()
    return nc


def _load_consts(g):
    nc, cp = g.nc, g.cp
    g.w_in_t = [cp.tile([128, E + 2 * ESH], BF16, tag=f"w_in{t}", name=f"w_in{t}") for t in range(2)]
    for t in range(2):
        nc.sync.dma_start(out=g.w_in_t[t][:], in_=g.w_in_d[t * 128:(t + 1) * 128, :])
    g.w_pw1_t = [cp.tile([128, MID], BF16, tag=f"w_pw1{t}", name=f"w_pw1{t}") for t in range(4)]
    for t in range(4):
        nc.sync.dma_start(out=g.w_pw1_t[t][:], in_=g.w_pw1_d[t * 128:(t + 1) * 128, :])
    g.pw1b_t = cp.tile([MID, 1], F32, tag="pw1b", name="pw1b")
    nc.sync.dma_start(out=g.pw1b_t[:], in_=g.pw1b_d[:])
    g.dwtap_t = cp.tile([MID, 9], F32, tag="dwtap", name="dwtap")
    nc.sync.dma_start(out=g.dwtap_t[:], in_=g.dwtap_d[:])
    g.w_pw2_t = cp.tile([MID, E], BF16, tag="w_pw2", name="w_pw2")
    nc.sync.dma_start(out=g.w_pw2_t[:], in_=g.w_pw2_d[:])
    g.w_xp_t = [cp.tile([128, 64], BF16, tag=f"w_xp{t}", name=f"w_xp{t}") for t in range(4)]
    for t in range(4):
        nc.sync.dma_start(out=g.w_xp_t[t][:], in_=g.w_xp_d[t * 128:(t + 1) * 128, :])
    g.w_dt_t = cp.tile([R, 2 * ESH], BF16, tag="w_dt", name="w_dt")
    nc.sync.dma_start(out=g.w_dt_t[:], in_=g.w_dt_d[:])
    g.spb_t = [cp.tile([ESH, 1], F32, tag=f"spb{c}", name=f"spb{c}") for c in range(2)]
    g.ascale_t = [cp.tile([2 * ESH, NJ], F32, tag=f"ascale{c}", name=f"ascale{c}") for c in range(2)]
    g.dire_t = [cp.tile([ESH, 4], F32, tag=f"dire{c}", name=f"dire{c}") for c in range(2)]
    g.dp4_t = [cp.tile([ESH, 1], F32, tag=f"dp4{c}", name=f"dp4{c}") for c in range(2)]
    g.dpb_t = [cp.tile([ESH, 1], F32, tag=f"dpb{c}", name=f"dpb{c}") for c in range(2)]
    g.w_out_t = [cp.tile([ESH, DM], BF16, tag=f"w_out{c}", name=f"w_out{c}") for c in range(2)]
    for c in range(2):
        nc.sync.dma_start(out=g.spb_t[c][:], in_=g.spb_d[c])
        nc.sync.dma_start(out=g.ascale_t[c][:], in_=g.ascale_d[c])
        nc.sync.dma_start(out=g.dire_t[c][:], in_=g.dire_d[c])
        nc.sync.dma_start(out=g.dp4_t[c][:], in_=g.dp4_d[c])
        nc.sync.dma_start(out=g.dpb_t[c][:], in_=g.dpb_d[c])
        nc.sync.dma_start(out=g.w_out_t[c][:], in_=g.w_out_d[c])
    g.sel_t = cp.tile([2 * ESH, ESH], BF16, tag="sel", name="sel")
    nc.sync.dma_start(out=g.sel_t[:], in_=g.sel_d[:])


def _front_stageA(g):
    """x+pos -> xz -> (z slices, xh) -> pw1 -> h12 (full [MID, L])."""
    nc, fp, fps = g.nc, g.fp, g.fps
    for (c0, nf) in MM_CHUNKS:
        xin = [fp.tile([128, 512], BF16, tag=f"xin{t}", bufs=2, name=f"xin{t}") for t in range(2)]
        for t in range(2):
            nc.sync.dma_start(out=xin[t][:, :nf],
                              in_=g.xT_d[t * 128:(t + 1) * 128, c0:c0 + nf])
        xhc = [fp.tile([128, 512], BF16, tag=f"xhc{m}", bufs=2, name=f"xhc{m}") for m in range(4)]
        for m in range(4):
            ps = fps.tile([128, 512], F32, tag="fps", bufs=2, name="ps_xz")
            for kt in range(2):
                nc.tensor.matmul(ps[:, :nf],
                                 lhsT=g.w_in_t[kt][:, m * 128:(m + 1) * 128],
                                 rhs=xin[kt][:, :nf],
                                 start=(kt == 0), stop=(kt == 1))
            nc.scalar.activation(out=xhc[m][:, :nf], in_=ps[:, :nf], func=AF.Copy)
        ps1 = fps.tile([MID, 512], F32, tag="fps", bufs=2, name="ps_pw1")
        for kt in range(4):
            nc.tensor.matmul(ps1[:, :nf], lhsT=g.w_pw1_t[kt][:],
                             rhs=xhc[kt][:, :nf],
                             start=(kt == 0), stop=(kt == 3))
        nc.scalar.activation(out=g.h12[:, c0:c0 + nf], in_=ps1[:, :nf],
                             func=AF.Identity, bias=g.pw1b_t[:])


def _stage_z(g):
    """z projection, deferred out of the head: re-DMAs x chunks (reusing
    stageA's xin buffers) and runs on the PE/Act slack under scan(cg0)."""
    nc, fp, fps = g.nc, g.fp, g.fps
    for (c0, nf) in MM_CHUNKS:
        xin = [fp.tile([128, 512], BF16, tag=f"xin{t}", bufs=2, name=f"zin{t}") for t in range(2)]
        for t in range(2):
            nc.sync.dma_start(out=xin[t][:, :nf],
                              in_=g.xT_d[t * 128:(t + 1) * 128, c0:c0 + nf])
        for cg in range(2):
            psz = fps.tile([ESH, 512], F32, tag="fps", bufs=2, name="ps_z")
            for kt in range(2):
                nc.tensor.matmul(psz[:, :nf],
                                 lhsT=g.w_in_t[kt][:, E + cg * ESH:E + (cg + 1) * ESH],
                                 rhs=xin[kt][:, :nf],
                                 start=(kt == 0), stop=(kt == 1))
            nc.scalar.activation(out=g.z_sl[cg][:, c0:c0 + nf], in_=psz[:, :nf],
                                 func=AF.Copy)


def _front_conv(g):
    """Depthwise 3x3 on h12 -> accB, in 4 row-bands so each band's ops
    start as soon as the h12 chunks covering its rows (+1 halo) land."""
    nc, fp = g.nc, g.fp
    acc = fp.tile([MID, L], BF16, tag="dwacc")
    acc3 = acc[:].rearrange("p (h w) -> p h w", w=W)
    h3 = g.h12[:].rearrange("p (h w) -> p h w", w=W)
    BAND = 12
    for b0 in range(0, H, BAND):
        b1 = b0 + BAND
        nc.vector.tensor_scalar(out=acc3[:, b0:b1, :],
                                in0=h3[:, b0:b1, :],
                                scalar1=g.dwtap_t[:, 4:5], scalar2=None,
                                op0=OP.mult)
        for ky in range(3):
            for kx in range(3):
                if ky == 1 and kx == 1:
                    continue
                dy, dx = ky - 1, kx - 1
                r0 = max(max(0, -dy), b0)
                r1 = min(H - max(0, dy), b1)
                if r0 >= r1:
                    continue
                w0, w1 = max(0, -dx), W - max(0, dx)
                nc.vector.scalar_tensor_tensor(
                    out=acc3[:, r0:r1, w0:w1],
                    in0=h3[:, r0 + dy:r1 + dy, w0 + dx:w1 + dx],
                    scalar=g.dwtap_t[:, ky * 3 + kx:ky * 3 + kx + 1],
                    in1=acc3[:, r0:r1, w0:w1],
                    op0=OP.mult, op1=OP.add)
        nc.scalar.activation(out=g.accB[:, b0 * W:b1 * W],
                             in_=acc[:, b0 * W:b1 * W], func=AF.Copy)


def _front_stageB(g):
    """pw2+SiLU -> xc chunks -> (xc_sl slice, x_dbl, delta, B/C to DRAM)."""
    nc, fp, fps = g.nc, g.fp, g.fps
    for (c0, nf) in MM_CHUNKS:
        xcc = [fp.tile([128, 512], BF16, tag=f"xcc{m}", bufs=2, name=f"xcc{m}") for m in range(4)]
        for m in range(4):
            ps2 = fps.tile([128, 512], F32, tag="fps", bufs=2, name="ps_pw2")
            nc.tensor.matmul(ps2[:, :nf],
                             lhsT=g.w_pw2_t[:, m * 128:(m + 1) * 128],
                             rhs=g.accB[:, c0:c0 + nf], start=True, stop=True)
            nc.scalar.activation(out=xcc[m][:, :nf], in_=ps2[:, :nf],
                                 func=AF.Silu)
        for cg in range(2):
            nc.sync.dma_start(out=g.xc_sl[cg][:, c0:c0 + nf],
                              in_=xcc[0][cg * ESH:(cg + 1) * ESH, :nf])
        ps3 = fps.tile([64, 512], F32, tag="fps", bufs=2, name="ps_xdbl")
        for kt in range(4):
            nc.tensor.matmul(ps3[:, :nf], lhsT=g.w_xp_t[kt][:],
                             rhs=xcc[kt][:, :nf],
                             start=(kt == 0), stop=(kt == 3))
        nc.scalar.activation(out=g.xdbl[:, c0:c0 + nf], in_=ps3[:, :nf],
                             func=AF.Copy)
        for cg in range(2):
            ps4 = fps.tile([ESH, 512], F32, tag="fps", bufs=2, name="ps_dt")
            nc.tensor.matmul(ps4[:, :nf],
                             lhsT=g.w_dt_t[:, cg * ESH:(cg + 1) * ESH],
                             rhs=g.xdbl[0:R, c0:c0 + nf], start=True, stop=True)
            nc.scalar.activation(out=g.dtlin[cg][:, c0:c0 + nf], in_=ps4[:, :nf],
                                 func=AF.Copy)
    # softplus(v) = ln(1 + exp(v)); |v| < ~10 here so exp cannot overflow.
    # Batched over full L to avoid ACT table reloads inside the chunk loop.
    for cg in range(2):
        nc.scalar.activation(out=g.delta_rep[cg][0:ESH, :], in_=g.dtlin[cg][:],
                             func=AF.Exp, bias=g.spb_t[cg][:])
        nc.scalar.activation(out=g.delta_rep[cg][0:ESH, :],
                             in_=g.delta_rep[cg][0:ESH, :], func=AF.Ln, bias=1.0)
        # duplicate delta rows [0:64] -> [64:128]
        nc.sync.dma_start(out=g.delta_rep[cg][ESH:2 * ESH, :],
                          in_=g.delta_rep[cg][0:ESH, :])
    # B/C rows to DRAM for later partition-broadcast loads
    nc.sync.dma_start(out=g.bsrc[:], in_=g.xdbl[R:R + N, :])
    nc.sync.dma_start(out=g.csrc[:], in_=g.xdbl[R + N:R + 2 * N, :])


def _scan_prep(g, cg):
    """y_acc init (D*u skip), fused-pair dA exps (with zero seam)."""
    nc, sp, wp = g.nc, g.sp, g.wp
    nc.scalar.activation(out=g.y_acc[cg][:], in_=g.xc_sl[cg][:],
                         func=AF.Identity, bias=g.dpb_t[cg][:],
                         scale=g.dp4_t[cg][:])
    for jp in range(NJ // 2):
        # dA for pair (2jp, 2jp+1) fused along free dim; the seam column
        # (fused position L = j-odd's t=0) is zeroed so the recurrence
        # restarts: h = 0*h_prev + dbu = the correct fresh-scan init.
        g.dA[jp] = sp.tile([128, 2 * L], BF16, tag=f"dA{jp}", name=f"dA{jp}")
        for s in range(2):
            nc.scalar.activation(out=g.dA[jp][:, s * L:(s + 1) * L],
                                 in_=g.delta_rep[cg][:], func=AF.Exp,
                                 scale=g.ascale_t[cg][:, 2 * jp + s:2 * jp + s + 1])
        # seam zero via Act (not DVE memset): stays on the Scalar queue
        # behind the two exps, so it never bubbles the DVE stream.
        nc.scalar.activation(out=g.dA[jp][:, L:L + 1],
                             in_=g.delta_rep[cg][:, 0:1],
                             func=AF.Identity, scale=0.0)



def _mk_du(g, cg, k):
    """u_k = perm_k(xc)+dir_k (Act); du = delta*u (DVE); ns-dup DMA.
    Emitted inside the previous direction's last pair so the dup DMA
    latency hides under that pair's hc work."""
    nc, wp = g.nc, g.wp
    xc3 = g.xc_sl[cg][:].rearrange("p (h w) -> p h w", w=W)
    xcT = g.xc_sl[cg][:].rearrange("p (h w) -> p w h", w=W)
    usrc = [xc3, xc3[:, ::-1, ::-1], xcT, xcT[:, ::-1, ::-1]][k]
    u_tmp = wp.tile([ESH, L], BF16, tag="u_tmp", bufs=1)
    u3 = u_tmp[:].rearrange("p (a c) -> p a c", c=W)
    nc.scalar.activation(out=u3, in_=usrc, func=AF.Identity,
                         bias=g.dire_t[cg][:, k:k + 1])
    du = wp.tile([128, L], BF16, tag="du", bufs=2)
    nc.vector.tensor_tensor(out=du[0:ESH, :], in0=g.delta_rep[cg][0:ESH, :],
                            in1=u_tmp[:], op=OP.mult)
    nc.sync.dma_start(out=du[ESH:2 * ESH, :], in_=du[0:ESH, :])
    return du


def _scan_k(g, cg, k, du, nxt=None, pending=None):
    """One direction's 8 scan tiles (scans fused in j-pairs). `nxt`
    (cg, k) identifies the following direction whose du block is emitted
    inside this direction's last pair. Returns (drain closure, next du)."""
    nc, wp, yps = g.nc, g.wp, g.yps
    du_next = None
    ypsum = [yps.tile([ESH, HALF], F32, tag=f"yps{h}", name=f"yps{h}") for h in range(2)]
    for jp in range(NJ // 2):
        dbu = wp.tile([128, 2 * L], BF16, tag="workA", bufs=1)
        for s in range(2):
            B_t = wp.tile([128, L], BF16, tag="B_t", bufs=2)
            C_s = [None, None]
            for ns in range(2):
                row = 4 * jp + 2 * s + ns
                nc.sync.dma_start(
                    out=B_t[ns * ESH:(ns + 1) * ESH, :],
                    in_=g.bsrc[row:row + 1, :].to_broadcast((ESH, L)))
            nc.vector.tensor_tensor(out=dbu[:, s * L:(s + 1) * L],
                                    in0=du[:], in1=B_t[:], op=OP.mult)
        h_t = wp.tile([128, 2 * L], BF16, tag="workH", bufs=1)
        nc.vector.tensor_tensor_scan(out=h_t[:], data0=g.dA[jp][:],
                                     data1=dbu[:], initial=0.0,
                                     op0=OP.mult, op1=OP.add)
        if jp == 0 and pending is not None:
            # previous direction's PSUM drain: emitted here (before this
            # direction's first start=True sel-matmul touches ypsum) but
            # after ~12us of fused dbu+scan, so it no longer stalls the
            # DVE on the PE's last sel-matmul of the previous direction.
            pending()
        if jp == NJ // 2 - 1 and nxt is not None:
            du_next = _mk_du(g, *nxt)
        for s in range(2):
            C_t = wp.tile([128, L], BF16, tag="C_t", bufs=2)
            for ns in range(2):
                row = 4 * jp + 2 * s + ns
                nc.sync.dma_start(
                    out=C_t[ns * ESH:(ns + 1) * ESH, :],
                    in_=g.csrc[row:row + 1, :].to_broadcast((ESH, L)))
            hc = wp.tile([128, L], BF16, tag="workB", bufs=2)
            nc.vector.tensor_tensor(out=hc[:], in0=h_t[:, s * L:(s + 1) * L],
                                    in1=C_t[:], op=OP.mult)
            for hh in range(2):
                for (c0, nf) in MM_CHUNKS_HALF:
                    nc.tensor.matmul(
                        ypsum[hh][:, c0:c0 + nf],
                        lhsT=g.sel_t[:],
                        rhs=hc[:, hh * HALF + c0:hh * HALF + c0 + nf],
                        start=(jp == 0 and s == 0),
                        stop=(jp == NJ // 2 - 1 and s == 1))
    def drain():
        # accumulate un-permuted ys_k into y_acc
        for hh in range(2):
            pv = ypsum[hh][:]
            if k == 0:
                dst = g.y_acc[cg][:, hh * HALF:(hh + 1) * HALF]
                srcv = pv
            elif k == 1:
                dst = g.y_acc[cg][:, (1 - hh) * HALF:(2 - hh) * HALF]
                srcv = pv[:, ::-1]
            elif k == 2:
                # ys[i], i=a*48+b_ -> l = b_*48+a ; half hh: a in [24hh,...)
                dst = g.y_acc[cg][:].rearrange("p (bb a) -> p bb a", a=W)[
                    :, :, 24 * hh:24 * hh + 24]
                srcv = pv.rearrange("p (a bb) -> p bb a", bb=W)
            else:
                dst = g.y_acc[cg][:].rearrange("p (bb a) -> p bb a", a=W)[
                    :, :, 24 * (1 - hh):24 * (1 - hh) + 24]
                srcv = pv.rearrange("p (a bb) -> p bb a", bb=W)[:, ::-1, ::-1]
            nc.vector.tensor_tensor(out=dst, in0=srcv, in1=dst, op=OP.add)
    return drain, du_next


def _finish_yv(g, cg):
    """yv[cg] = y_acc * silu(z) for this channel group."""
    nc, wp = g.nc, g.wp
    sz = wp.tile([ESH, L], BF16, tag="u_tmp", bufs=1)
    nc.scalar.activation(out=sz[:], in_=g.z_sl[cg][:], func=AF.Silu)
    g.yv[cg] = g.fp.tile([ESH, L], BF16, tag=f"dtlin{cg}", name=f"yv{cg}")
    nc.vector.tensor_tensor(out=g.yv[cg][:], in0=g.y_acc[cg][:], in1=sz[:],
                            op=OP.mult)


def _out_proj(g):
    """out_partial = sum_cg W_out[cg]^T @ yv[cg] (PSUM-accumulated)."""
    nc, wp, fps = g.nc, g.wp, g.fps
    for m in range(2):
        for (c0, nf) in MM_CHUNKS:
            po = fps.tile([128, 512], F32, tag="fps", bufs=2, name="ps_out")
            for cg in range(2):
                nc.tensor.matmul(po[:, :nf],
                                 lhsT=g.w_out_t[cg][:, m * 128:(m + 1) * 128],
                                 rhs=g.yv[cg][:, c0:c0 + nf],
                                 start=(cg == 0), stop=(cg == 1))
            osb = wp.tile([128, 512], F32, tag="osb", bufs=2)
            nc.scalar.activation(out=osb[:, :nf], in_=po[:, :nf], func=AF.Copy)
            nc.sync.dma_start(out=g.out_d[m * 128:(m + 1) * 128, c0:c0 + nf],
                              in_=osb[:, :nf])


def _r32r(a):
    """Round fp32 -> fp32r (TF32-like, 10 explicit mantissa bits)."""
    b = np.ascontiguousarray(a, np.float32).view(np.uint32)
    return (((b.astype(np.uint64) + 0x1000) & 0xFFFFE000)
            .astype(np.uint32).view(np.float32))


def _bf16(a):
    return np.ascontiguousarray(np.asarray(a, np.float32)).astype(
        ml_dtypes.bfloat16)


def _host_prep(inputs):
    x = np.asarray(inputs["x"], np.float32)
    W_pos = np.asarray(inputs["W_pos"], np.float32)
    b_pos = np.asarray(inputs["b_pos"], np.float32)
    W_in = np.asarray(inputs["W_in"], np.float32)
    pw1_w = np.asarray(inputs["pw1_w"], np.float32)
    pw1_b = np.asarray(inputs["pw1_b"], np.float32)
    dw_w = np.asarray(inputs["dw_w"], np.float32)
    pw2_w = np.asarray(inputs["pw2_w"], np.float32)
    W_xproj = np.asarray(inputs["W_xproj"], np.float32)
    W_dt = np.asarray(inputs["W_dt"], np.float32)
    b_dt = np.asarray(inputs["b_dt"], np.float32)
    A_log = np.asarray(inputs["A_log"], np.float32)
    Dp = np.asarray(inputs["Dp"], np.float32)
    dir_emb = np.asarray(inputs["dir_emb"], np.float32)
    W_out = np.asarray(inputs["W_out"], np.float32)

    gy, gx = np.meshgrid(np.arange(H, dtype=np.float32),
                         np.arange(W, dtype=np.float32), indexing="ij")
    coords = np.stack([gy, gx], -1) / (H - 1) * 2 - 1
    pos = (coords.reshape(L, 2) @ W_pos + b_pos).astype(np.float32)

    common = {
        "w_pw1": _bf16(pw1_w.reshape(MID, E).T),
        "pw1b": np.ascontiguousarray(pw1_b.reshape(MID, 1)),
        "dwtap": np.ascontiguousarray(dw_w.reshape(MID, 9)),
    }
    w_pw2_base = pw2_w.reshape(E, MID).T  # (MID, E)
    A = -np.exp(A_log)  # (E, N)
    xp = (x + pos[None]).transpose(0, 2, 1)  # (B, Dm, L)

    sel = np.zeros((2 * ESH, ESH), np.float32)
    for p in range(2 * ESH):
        sel[p, p % ESH] = 1.0
    sel = sel.astype(ml_dtypes.bfloat16)

    in_maps = []
    for c in range(NCORES):
        bcr = c // 4               # this core's batch
        e0 = (c % 4) * 2 * ESH     # this core's 128-channel slice
        sl = slice(e0, e0 + 2 * ESH)
        ascale = np.empty((2, 2 * ESH, NJ), np.float32)
        for cg in range(2):
            A_cg = A[e0 + cg * ESH:e0 + (cg + 1) * ESH]  # (64, 16)
            for p in range(2 * ESH):
                for j in range(NJ):
                    ascale[cg, p, j] = A_cg[p % ESH, 2 * j + p // ESH]
        m = dict(common)
        m["xT"] = _bf16(xp[bcr])
        # channel permutation putting this core's slice at rows [0:128]
        perm = np.concatenate([np.arange(e0, e0 + 2 * ESH),
                               np.arange(0, e0),
                               np.arange(e0 + 2 * ESH, E)])
        m["w_pw2"] = _bf16(w_pw2_base[:, perm])
        m["w_xp"] = _bf16(np.concatenate(
            [W_xproj[perm, :], np.zeros((E, 64 - (R + 2 * N)), np.float32)],
            axis=1))
        m["w_in"] = _bf16(
            np.concatenate([W_in[:, :E], W_in[:, E + e0:E + e0 + 2 * ESH]],
                           axis=1))
        m["w_dt"] = _bf16(W_dt[:, sl])
        m["spb"] = np.ascontiguousarray(
            (2.0 * b_dt[sl]).reshape(2, ESH, 1))
        m["ascale"] = ascale
        m["dire"] = np.ascontiguousarray(
            dir_emb[:, sl].T.reshape(2, ESH, 4))
        m["dp4"] = np.ascontiguousarray((4.0 * Dp[sl]).reshape(2, ESH, 1))
        m["dpb"] = np.ascontiguousarray(
            (Dp[sl] * dir_emb[:, sl].sum(0)).reshape(2, ESH, 1))
        m["w_out"] = _bf16(W_out[sl, :].reshape(2, ESH, DM))
        m["sel"] = sel
        in_maps.append(m)
    return in_maps


_PROGRAM = None
_LAST_RESULTS = None
_LAST_INSTS = None


def _get_program():
    global _PROGRAM
    if _PROGRAM is None:
        _PROGRAM = build_program()
    return _PROGRAM


def kernel(**inputs):
    global _LAST_EXEC_NS, _LAST_RESULTS
    assert int(inputs["H"]) == H and int(inputs["W"]) == W
    in_maps = _host_prep(inputs)
    if TRACE:
        _install_profile_shim()
    res = run_bass_kernel_spmd(_get_program(), in_maps,
                               list(range(NCORES)), trace=TRACE)
    _LAST_EXEC_NS = res.exec_time_ns
    _LAST_RESULTS = res.results
    global _LAST_INSTS
    _LAST_INSTS = res.instructions_and_trace
    out = np.zeros((B, DM, L), np.float32)
    for c, r in enumerate(res.results):
        out[c // 4] += np.asarray(r["out"], np.float32)
    return np.ascontiguousarray(out.transpose(0, 2, 1))
